# revision 69
# baseline (speedup 1.0000x reference)
"""Trainium2 Bass kernel for nn_MultiHeadAttention (B=2, S=2048, E=1024, H=16).

Sharding: 8 NeuronCores = data-parallel over the 2 batches x tensor-parallel
over the 16 heads in 4 groups of 4 heads (Wq/Wk/Wv split column-wise, Wo
row-wise).  Each core computes a full-[S, E] partial of its batch's output;
the host sums the 4 head-group partials per batch.

Per-core device algorithm (S.T orientation feeds a flipped P@V):
  Q.T/K.T[n, s] = (wT chunk).T @ xT chunk       e/m-outer projections
  S.T_h[k, q]   = (K_h.T chunk).T @ Q_h.T       row-packed head pairs (d=64)
  P.T           = exp(S.T / 8)                  fp16, one ACT op per (k, pair)
  O[q, d|sum]   = (P.T chunk).T @ [V_h | 1]     FLIPPED: P.T [128k,128q] is the
                                                stationary, [V|ones] [128k,65]
                                                moving -> 65-col outputs, with
                                                the softmax denominator landing
                                                in column 64 (per-partition!)
  O_n           = O * recip(col 64)             one DVE tensor_scalar per tile
  O.T           = PE-transpose(O_n)             53ns/tile, restores [d, q]
  out[m, :]     = sum_pair (oT2 chunk).T @ woT2 contract-128 output projection

The flip + contract-128 out-proj cut PE matmul time ~27%; exp on the
Activation engine (~133us) becomes the critical resource, so the instruction
stream is interleaved to keep it saturated: each (qc, pair) "window" weaves
the NEXT window's scores+exp with THIS window's PV, plus V/Q-projection,
out-projection, and transpose filler work sized to the Act-engine pace.

dtypes: HBM traffic fp16 (in+out); scores fp32r; P/V/O/Wo fp16; accum fp32.
"""

import numpy as np
from contextlib import ExitStack

import ml_dtypes

import concourse.bass as bass
import concourse.mybir as mybir
import concourse.tile as tile
from concourse.tile import ScopedClock
from concourse.bass_utils import run_bass_kernel_spmd

# ---------------------------------------------------------------------------
# Workarounds for the walrus build on this stack, which rejects more than ONE
# semaphore wait per instruction ("Too many sync wait commands").
# ---------------------------------------------------------------------------
_orig_commit_instruction = tile.TileContext._commit_instruction


def _commit_instruction(self, inst, lazy_reg_writes=True):
    si = getattr(inst, "sync_info", None)
    if si is not None and si.on_wait and len(si.on_wait) > 1:
        waits = list(si.on_wait)
        for w in waits[:-1]:
            nop = mybir.InstNoOp(
                name=self.nc.get_next_instruction_name(),
                ins=[], outs=[], engine=inst.engine,
            )
            nop.bass_nofuse = True
            nop.sync_info = mybir.SyncInfo(on_wait=[w], on_update=[])
            _orig_commit_instruction(self, nop, lazy_reg_writes=False)
        inst.sync_info = mybir.SyncInfo(
            on_wait=[waits[-1]], on_update=list(si.on_update or [])
        )
    return _orig_commit_instruction(self, inst, lazy_reg_writes)


def _drain_and_barrier(self, tick_clock, wait_clock):
    nc = self.nc
    drain_inst = nc.sync.drain()
    wait_clock.add_sem_waits(
        drain_inst.ins, ScopedClock({None: tick_clock.global_clock})
    )
    si = drain_inst.ins.sync_info
    waits = list(si.on_wait) if si and si.on_wait else []
    if len(waits) > 1:
        drain_inst.ins.sync_info = mybir.SyncInfo(
            on_wait=waits[:1], on_update=list(si.on_update or [])
        )
        for w in waits[1:]:
            extra = nc.sync.drain()
            esi = extra.ins.sync_info
            extra.ins.sync_info = mybir.SyncInfo(
                on_wait=[w],
                on_update=list(esi.on_update or []) if esi else [],
            )
    nc.all_engine_barrier()
    assert self.sems is not None
    popped = nc._tile_sem_poison_stack.pop()
    assert popped is self._sem_poison
    nc.clear_and_free_semaphores(list(self.sems.allocated().values()))
    nc.all_engine_barrier()


def _apply_tilefix():
    tile.TileContext._commit_instruction = _commit_instruction
    tile.TileContext._drain_and_barrier = _drain_and_barrier


_apply_tilefix()

# ---------------------------------------------------------------------------
# Problem constants (hardcoded)
# ---------------------------------------------------------------------------
B, S, E, H = 2, 2048, 1024, 16
HC, D = 4, 64              # heads per core, head dim
NCORES = 8
NE = E // 128              # 8  e-chunks (projection contraction)
NK = S // 128              # 16 k-chunks
NQC = S // 512             # 4  512-wide q windows
NM = S // 128              # 16 output row chunks

F32 = mybir.dt.float32
F32R = mybir.dt.float32r
FP16 = mybir.dt.float16
EXP = mybir.ActivationFunctionType.Exp
MULT = mybir.AluOpType.mult


def build(ptbufs=20, xdt=FP16, qkdt=mybir.dt.float32r):
    nc = bass.Bass()
    xqT = nc.dram_tensor("xqT", [E, S], xdt, kind="ExternalInput")
    xkT = nc.dram_tensor("xkT", [E, S], xdt, kind="ExternalInput")
    xvT = nc.dram_tensor("xvT", [E, S], xdt, kind="ExternalInput")
    wqT = nc.dram_tensor("wqT", [E, 256], xdt, kind="ExternalInput")
    wkT = nc.dram_tensor("wkT", [E, 256], xdt, kind="ExternalInput")
    wvT = nc.dram_tensor("wvT", [E, 256], xdt, kind="ExternalInput")
    woT2 = nc.dram_tensor("woT2", [128, 2 * E], FP16, kind="ExternalInput")
    ident = nc.dram_tensor("ident", [128, 128], FP16, kind="ExternalInput")
    out = nc.dram_tensor("out", [S, E], FP16, kind="ExternalOutput")

    with tile.TileContext(nc) as tc, ExitStack() as ctx:
        consts = ctx.enter_context(tc.tile_pool(name="consts", bufs=1))
        wpool = ctx.enter_context(tc.tile_pool(name="w", bufs=1))
        actpool = ctx.enter_context(tc.tile_pool(name="acts", bufs=1))
        xkvpool = ctx.enter_context(tc.tile_pool(name="xkv", bufs=10))
        xqpool = ctx.enter_context(tc.tile_pool(name="xq", bufs=4))

        # preload the exp table before the hot loop
        dummy = consts.tile([1, 8], F32)
        nc.vector.memset(dummy[:], 0.0)
        nc.scalar.activation(dummy[:], dummy[:], EXP)

        wq_sb = wpool.tile([128, NE, 256], xdt)
        wk_sb = wpool.tile([128, NE, 256], xdt)
        wv_sb = wpool.tile([128, NE, 256], xdt)
        wo2_sb = wpool.tile([128, 2, E], FP16)
        id_sb = wpool.tile([128, 128], FP16)

        # per-pair tensors: tile-granular dependency tracking means a read
        # waits on every writer emitted so far, so pair0's scores must not
        # share a tile with pair1's (later-copied) projections
        qT_sb = [actpool.tile([128, S], qkdt, name=f"qT{p}") for p in range(2)]
        kT_sb = [actpool.tile([128, S], qkdt, name=f"kT{p}") for p in range(2)]
        v_sb = actpool.tile([128, NK, HC, 65], FP16)   # [s%128, k, h, V_h|one]
        oT2_sb = actpool.tile([128, 2, S], FP16)       # [(2 heads x d), pair, q]

        # ---- DMA program: ordered by first-use ----
        nc.sync.dma_start(wk_sb[:], wkT.rearrange("(ec p) n -> p ec n", p=128))
        nc.sync.dma_start(wq_sb[:], wqT.rearrange("(ec p) n -> p ec n", p=128))
        xk = []
        for e in range(NE):
            t = xkvpool.tile([128, S], xdt, tag="xc", name=f"xk{e}")
            nc.sync.dma_start(t[:], xkT[e * 128:(e + 1) * 128, :])
            xk.append(t)
        # xq arrives in per-qc column blocks (ONE descriptor each: HWDGE
        # descriptor-gen at ~625ns/DMA would otherwise pace the 364ns slices)
        xqv = xqT.rearrange("(ec p) s -> p ec s", p=128)
        xq = [None] * NQC
        for qc in [0]:
            t = xqpool.tile([128, NE, 512], xdt, tag="xq", name=f"xq{qc}")
            nc.sync.dma_start(t[:], xqv[:, :, qc * 512:(qc + 1) * 512])
            xq[qc] = t
        xv = []
        for e in range(NE):
            t = xkvpool.tile([128, S], xdt, tag="xc", name=f"xv{e}")
            nc.sync.dma_start(t[:], xvT[e * 128:(e + 1) * 128, :])
            xv.append(t)
        # wv rides behind xv (first use ~32us, xv gates the V projections)
        nc.sync.dma_start(wv_sb[:], wvT.rearrange("(ec p) n -> p ec n", p=128))
        for qc in range(1, NQC):
            if qc == 2:
                # wo2/ident ride between xq1 and xq2 (first use ~55us); they
                # must not delay xq1, which gates the qc1 Q-projection
                nc.sync.dma_start(
                    wo2_sb[:], woT2.rearrange("p (two e) -> p two e", two=2))
                nc.sync.dma_start(id_sb[:], ident[0:128, :])
            t = xqpool.tile([128, NE, 512], xdt, tag="xq", name=f"xq{qc}")
            nc.sync.dma_start(t[:], xqv[:, :, qc * 512:(qc + 1) * 512])
            xq[qc] = t

        # ones column for the fused rowsum (PV moving operand col 64)
        nc.vector.memset(v_sb[:, :, :, 64:65], 1.0)

        # ---- steady-state pools (psS first: it coexists with prefix psA) ----
        psS = ctx.enter_context(tc.tile_pool(name="psS", bufs=2, space="PSUM"))
        ppool = ctx.enter_context(tc.tile_pool(name="pT", bufs=ptbufs))
        npool = ctx.enter_context(tc.tile_pool(name="nT", bufs=6))
        rpool = ctx.enter_context(tc.tile_pool(name="rt", bufs=4))
        stpool = ctx.enter_context(tc.tile_pool(name="stage", bufs=4))

        def emit_scores_exp(qc, pair, k):
            qs = slice(qc * 512, (qc + 1) * 512)
            ks = slice(k * 128, (k + 1) * 128)
            ps_s = psS.tile([128, 1024], F32, tag="ss", name=f"ss{qc}_{pair}_{k}")
            nc.tensor.matmul(ps_s[:, 0:512],
                             kT_sb[pair][0:64, ks], qT_sb[pair][0:64, qs],
                             start=True, stop=True)
            nc.tensor.matmul(ps_s[:, 512:1024],
                             kT_sb[pair][64:128, ks], qT_sb[pair][64:128, qs],
                             start=True, stop=True)
            pT = ppool.tile([128, 1024], FP16, tag="pt", name=f"pt{qc}_{pair}_{k}")
            nc.scalar.activation(pT[:], ps_s[:], EXP, scale=0.125)
            return pT

        def emit_pv(pair, k, pT, ovv, qts=(0, 1, 2, 3)):
            # ovv: ps_o viewed as [128, qt, h2, 128]; O in cols 0:64, sum col
            # 64. qt-major so qt 0-1 fill PSUM bank0 and qt 2-3 bank1 (the
            # last window is drained per qt-half). PSUM zero regions are 2KB:
            # start=True pending-zeroes the WHOLE bank, so only the first
            # region per bank may start the group; the other regions' k==0
            # writes land on pending-zero bytes and write-through (hardware
            # lazy-zero semantics).
            for qt in qts:
                for h2 in range(2):
                    h = pair * 2 + h2
                    nc.tensor.matmul(
                        ovv[:, qt, h2, 0:65],
                        pT[:, h2 * 512 + qt * 128:h2 * 512 + (qt + 1) * 128],
                        v_sb[:, k, h, 0:65],
                        start=(k == 0 and h2 == 0 and qt in (0, 2)),
                        stop=(k == NK - 1),
                        skip_group_check=True,
                    )

        def emit_vproj(k):
            t = psM.tile([128, 512], F32, tag="mi", name=f"vp{k}")
            for e in range(NE):
                nc.tensor.matmul(
                    t[:, 0:256],
                    xv[e][:, k * 128:(k + 1) * 128],
                    wv_sb[:, e, :],
                    start=(e == 0), stop=(e == NE - 1),
                )
            nc.vector.tensor_copy(
                v_sb[:, k, :, 0:64],
                t[:, 0:256].rearrange("p (h c) -> p h c", h=HC))

        def emit_qproj(qc, nch):
            t = psM.tile([128, 512], F32, tag="mi", name=f"qp{qc}_{nch}")
            for e in range(NE):
                nc.tensor.matmul(
                    t[:],
                    wq_sb[:, e, nch * 128:(nch + 1) * 128],
                    xq[qc][:, e, :],
                    start=(e == 0), stop=(e == NE - 1),
                )
            nc.vector.tensor_copy(qT_sb[nch][:, qc * 512:(qc + 1) * 512], t[:])

        def emit_norm(qc, pair, ovv, qts=(0, 1, 2, 3), split=False):
            # reciprocal of the fused rowsums (col 64 of each 128-col region)
            n = len(qts)
            rt = rpool.tile([128, 2 * n], F32, tag="rt",
                            name=f"rt{qc}_{pair}_{qts[0]}")
            nc.vector.reciprocal(
                rt[:].rearrange("p (q h) -> p q h", q=n),
                ovv[:, qts[0]:qts[0] + n, :, 64])
            nTs = {}
            for i, qt in enumerate(qts):
                nT = npool.tile([128, 128], FP16, tag="nt",
                                name=f"nt{qc}_{pair}_{qt}")
                for h2 in range(2):
                    if split and qt >= 2:
                        # tail only: whole tiles on the idle Act engine
                        # (mixed engines per tile serialize on WAW sems)
                        nc.scalar.mul(nT[:, h2 * 64:(h2 + 1) * 64],
                                      ovv[:, qt, h2, 0:64],
                                      rt[:, 2 * i + h2:2 * i + h2 + 1])
                    else:
                        nc.vector.tensor_scalar(
                            nT[:, h2 * 64:(h2 + 1) * 64],
                            ovv[:, qt, h2, 0:64],
                            rt[:, 2 * i + h2:2 * i + h2 + 1],
                            None, MULT)
                nTs[qt] = nT
            return nTs

        def emit_transpose(qc, pair, qt, nT, pool=None):
            # pool=psS in the tail: scores are done, its slots are idle, and
            # keeping transposes out of the psM ring stops them serializing
            # behind out-proj tiles awaiting their stage copies.
            pool = pool or psM
            tag = {id(psS): "ss", id(psOV): "ov"}.get(id(pool), "mi")
            psT = pool.tile([128, 128], FP16, tag=tag, name=f"tp{qc}_{pair}_{qt}")
            nc.tensor.transpose(psT[:], nT[:], id_sb[:])
            nc.vector.tensor_copy(
                oT2_sb[:, pair, qc * 512 + qt * 128:qc * 512 + (qt + 1) * 128],
                psT[:])

        def emit_outproj(m, split=False, pool=None):
            pool = pool or psM
            tag = {id(psS): "ss", id(psOV): "ov"}.get(id(pool), "mi")
            stage = stpool.tile([128, E], FP16, tag="st", name=f"st{m}")
            for j in range(2):
                t = pool.tile([128, 512], F32, tag=tag, name=f"op{m}_{j}")
                for pair in range(2):
                    nc.tensor.matmul(
                        t[:],
                        oT2_sb[:, pair, m * 128:(m + 1) * 128],
                        wo2_sb[:, pair, j * 512:(j + 1) * 512],
                        start=(pair == 0), stop=(pair == 1),
                    )
                if split and j == 1:
                    nc.scalar.copy(stage[:, 512:1024], t[:])
                else:
                    nc.vector.tensor_copy(stage[:, j * 512:(j + 1) * 512], t[:])
            if split:
                nc.sync.dma_start(out[m * 128:(m + 1) * 128, :], stage[:])
            else:
                nc.gpsimd.dma_start(out[m * 128:(m + 1) * 128, :], stage[:])

        # ---- prefix: K/Q projections, nch-split so pair0 lands fast ----
        # psA (4 banks) coexists with psS (4 banks); pair0's K tiles project
        # and copy out first, Q-qc0 goes through psS slots, and the first
        # scores fire while pair1's K projection finishes in the background.
        pts = {}
        cur = []
        vdone = 0
        with tc.tile_pool(name="psA", bufs=4, space="PSUM") as psA:
            kt0 = [psA.tile([128, 512], F32, tag="mm", name=f"pk0_{m}")
                   for m in range(4)]
            kt1 = [psA.tile([128, 512], F32, tag="mm", name=f"pk1_{m}")
                   for m in range(4)]
            for e in range(NE):
                for m in range(4):
                    nc.tensor.matmul(
                        kt0[m][:], wk_sb[:, e, 0:128],
                        xk[e][:, m * 512:(m + 1) * 512],
                        start=(e == 0), stop=(e == NE - 1))
            for m in range(4):
                nc.vector.tensor_copy(
                    kT_sb[0][:, m * 512:(m + 1) * 512], kt0[m][:])
            # Q-qc0 pair0 chases the xq0 DMAs immediately after K-pair0 (the
            # PE stream is in-order, so nothing may sit between them); all of
            # pair1's projection work weaves into the first scores' shadows.
            tq0 = psS.tile([128, 512], F32, tag="ss", name="pq0_0")
            # tq1 must NOT take a psS ring slot: its reader (the qT1 copy)
            # lands after the early scores, which would deadlock their
            # tile allocations. The psA ring's kt0[0] slot frees early.
            tq1 = psA.tile([128, 512], F32, tag="mm", name="pq0_1")
            for e in range(NE):
                nc.tensor.matmul(
                    tq0[:], wq_sb[:, e, 0:128], xq[0][:, e, :],
                    start=(e == 0), stop=(e == NE - 1))
            nc.vector.tensor_copy(qT_sb[0][:, 0:512], tq0[:])
            for k in range(6):
                cur.append(emit_scores_exp(0, 0, k))
                if k < 4:     # K-pair1, two e-chunks per score shadow
                    for e in (2 * k, 2 * k + 1):
                        for m in range(4):
                            nc.tensor.matmul(
                                kt1[m][:], wk_sb[:, e, 128:256],
                                xk[e][:, m * 512:(m + 1) * 512],
                                start=(e == 0), stop=(e == NE - 1))
                elif k < 6:   # then Q-pair1, four e-chunks per shadow
                    for e in range(4 * (k - 4), 4 * (k - 3)):
                        nc.tensor.matmul(
                            tq1[:], wq_sb[:, e, 128:256], xq[0][:, e, :],
                            start=(e == 0), stop=(e == NE - 1))
            for m in range(4):
                nc.vector.tensor_copy(
                    kT_sb[1][:, m * 512:(m + 1) * 512], kt1[m][:])
            for k in range(6, 8):
                cur.append(emit_scores_exp(0, 0, k))
                if k == 6:
                    nc.vector.tensor_copy(qT_sb[1][:, 0:512], tq1[:])
        psOV = ctx.enter_context(tc.tile_pool(name="psOV", bufs=1, space="PSUM"))
        psM = ctx.enter_context(tc.tile_pool(name="psM", bufs=2, space="PSUM"))

        for k in range(8, NK):
            cur.append(emit_scores_exp(0, 0, k))
            if k >= 9 and vdone < 5:   # xv DMAs have landed by these shadows
                emit_vproj(vdone)
                vdone += 1
            elif k == 14:
                emit_qproj(1, 0)
            elif k == 15:
                emit_qproj(1, 1)
        pts[(0, 0)] = cur

        seq = [(qc, pair) for qc in range(NQC) for pair in range(2)]
        trans_pending = None   # (qc, pair, nTs) awaiting transpose weave
        for i, (qc, pair) in enumerate(seq):
            nxt = seq[i + 1] if i + 1 < len(seq) else None
            ps_o = psOV.tile([128, 1024], F32, tag="ov", name=f"ov{qc}_{pair}")
            ovv = ps_o.rearrange("p (q h c) -> p q h c", q=4, h=2)
            cur_pts = pts.pop((qc, pair))
            nxt_pts = [] if nxt else None
            op_ms = list(range((qc - 1) * 4, qc * 4)) if (pair == 0 and qc >= 1) else []
            last_qts = (0, 1, 2, 3)
            for k in range(NK):
                if nxt:
                    nxt_pts.append(emit_scores_exp(nxt[0], nxt[1], k))
                if trans_pending and k in (2, 4, 6, 8):
                    tqc, tpair, tnTs = trans_pending
                    emit_transpose(tqc, tpair, k // 2 - 1, tnTs[k // 2 - 1])
                    if k == 8:
                        trans_pending = None
                if vdone < NK:  # remaining V-proj tiles, just-in-time
                    emit_vproj(vdone)
                    vdone += 1
                emit_pv(pair, k, cur_pts[k], ovv, qts=last_qts)
                if op_ms and k % 4 == 2:
                    emit_outproj(op_ms[k // 4])
                if pair == 0 and 1 <= qc < NQC - 1 and k in (8, 12):
                    # Q projection for the next qc, before its scores appear
                    emit_qproj(qc + 1, (k - 8) // 4)
            if nxt:
                pts[nxt] = nxt_pts
                nTs = emit_norm(qc, pair, ovv)
                trans_pending = (qc, pair, nTs)
            else:
                # tail drain: norm (split across DVE + the idle Act engine),
                # then per-q-tile transpose + out-proj chains distributed
                # over all three PSUM rings so they pipeline
                mb = (NQC - 1) * 4
                nTs = emit_norm(qc, pair, ovv, split=True)
                emit_transpose(qc, pair, 0, nTs[0], pool=psS)
                emit_transpose(qc, pair, 1, nTs[1], pool=psS)
                emit_transpose(qc, pair, 2, nTs[2], pool=psOV)
                emit_transpose(qc, pair, 3, nTs[3], pool=psOV)
                emit_outproj(mb + 0, split=True, pool=psM)
                emit_outproj(mb + 1, split=True, pool=psS)
                emit_outproj(mb + 2, split=True, pool=psM)
                emit_outproj(mb + 3, split=True, pool=psS)

    return nc


_NC_CACHE = {}


def _get_nc():
    if "nc" not in _NC_CACHE:
        _NC_CACHE["nc"] = build()
    return _NC_CACHE["nc"]


def _shard_inputs(query, key, value, Wq, Wk, Wv, Wo):
    """Host-side sharding + layout prep: core c = (batch c//4, head-group c%4)."""
    f16 = np.float16
    xT = []
    for b in range(B):
        xT.append((
            np.ascontiguousarray(query[b].T).astype(f16),
            np.ascontiguousarray(key[b].T).astype(f16),
            np.ascontiguousarray(value[b].T).astype(f16),
        ))
    wT = []
    for g in range(4):
        gc = slice(g * 256, (g + 1) * 256)
        wo_g = Wo[:, gc].T.astype(f16)            # [256, E]
        woT2 = np.ascontiguousarray(
            wo_g.reshape(2, 128, E).transpose(1, 0, 2).reshape(128, 2 * E))
        wT.append((
            np.ascontiguousarray(Wq[gc].T).astype(f16),
            np.ascontiguousarray(Wk[gc].T).astype(f16),
            np.ascontiguousarray(Wv[gc].T).astype(f16),
            woT2,
        ))
    ident = np.eye(128, dtype=f16)
    in_maps = []
    for c in range(NCORES):
        b, g = c // 4, c % 4
        qT, kT, vT = xT[b]
        wq, wk, wv, wo2 = wT[g]
        in_maps.append({
            "xqT": qT, "xkT": kT, "xvT": vT,
            "wqT": wq, "wkT": wk, "wvT": wv, "woT2": wo2,
            "ident": ident,
        })
    return in_maps


def kernel(query, key, value, Wq, Wk, Wv, Wo):
    query = np.asarray(query, dtype=np.float32)
    key = np.asarray(key, dtype=np.float32)
    value = np.asarray(value, dtype=np.float32)
    Wq = np.asarray(Wq, dtype=np.float32)
    Wk = np.asarray(Wk, dtype=np.float32)
    Wv = np.asarray(Wv, dtype=np.float32)
    Wo = np.asarray(Wo, dtype=np.float32)

    nc = _get_nc()
    in_maps = _shard_inputs(query, key, value, Wq, Wk, Wv, Wo)
    res = run_bass_kernel_spmd(nc, in_maps, core_ids=list(range(NCORES)))

    out = np.zeros((B, S, E), dtype=np.float32)
    for c in range(NCORES):
        out[c // 4] += res.results[c]["out"].astype(np.float32)
    return out


# revision 70
# speedup vs baseline: 1.0109x; 1.0109x over previous
"""Trainium2 Bass kernel for nn_MultiHeadAttention (B=2, S=2048, E=1024, H=16).

Sharding: 8 NeuronCores = data-parallel over the 2 batches x tensor-parallel
over the 16 heads in 4 groups of 4 heads (Wq/Wk/Wv split column-wise, Wo
row-wise).  Each core computes a full-[S, E] partial of its batch's output;
the host sums the 4 head-group partials per batch.

Per-core device algorithm (S.T orientation feeds a flipped P@V):
  Q.T/K.T[n, s] = (wT chunk).T @ xT chunk       e/m-outer projections
  S.T_h[k, q]   = (K_h.T chunk).T @ Q_h.T       row-packed head pairs (d=64)
  P.T           = exp(S.T / 8)                  fp16, one ACT op per (k, pair)
  O[q, d|sum]   = (P.T chunk).T @ [V_h | 1]     FLIPPED: P.T [128k,128q] is the
                                                stationary, [V|ones] [128k,65]
                                                moving -> 65-col outputs, with
                                                the softmax denominator landing
                                                in column 64 (per-partition!)
  O_n           = O * recip(col 64)             one DVE tensor_scalar per tile
  O.T           = PE-transpose(O_n)             53ns/tile, restores [d, q]
  out[m, :]     = sum_pair (oT2 chunk).T @ woT2 contract-128 output projection

The flip + contract-128 out-proj cut PE matmul time ~27%; exp on the
Activation engine (~133us) becomes the critical resource, so the instruction
stream is interleaved to keep it saturated: each (qc, pair) "window" weaves
the NEXT window's scores+exp with THIS window's PV, plus V/Q-projection,
out-projection, and transpose filler work sized to the Act-engine pace.

dtypes: HBM traffic fp16 (in+out); scores fp32r; P/V/O/Wo fp16; accum fp32.
"""

import numpy as np
from contextlib import ExitStack

import ml_dtypes

import concourse.bass as bass
import concourse.mybir as mybir
import concourse.tile as tile
from concourse.tile import ScopedClock
from concourse.bass_utils import run_bass_kernel_spmd

# ---------------------------------------------------------------------------
# Workarounds for the walrus build on this stack, which rejects more than ONE
# semaphore wait per instruction ("Too many sync wait commands").
# ---------------------------------------------------------------------------
_orig_commit_instruction = tile.TileContext._commit_instruction


def _commit_instruction(self, inst, lazy_reg_writes=True):
    si = getattr(inst, "sync_info", None)
    if si is not None and si.on_wait and len(si.on_wait) > 1:
        waits = list(si.on_wait)
        for w in waits[:-1]:
            nop = mybir.InstNoOp(
                name=self.nc.get_next_instruction_name(),
                ins=[], outs=[], engine=inst.engine,
            )
            nop.bass_nofuse = True
            nop.sync_info = mybir.SyncInfo(on_wait=[w], on_update=[])
            _orig_commit_instruction(self, nop, lazy_reg_writes=False)
        inst.sync_info = mybir.SyncInfo(
            on_wait=[waits[-1]], on_update=list(si.on_update or [])
        )
    return _orig_commit_instruction(self, inst, lazy_reg_writes)


def _drain_and_barrier(self, tick_clock, wait_clock):
    nc = self.nc
    drain_inst = nc.sync.drain()
    wait_clock.add_sem_waits(
        drain_inst.ins, ScopedClock({None: tick_clock.global_clock})
    )
    si = drain_inst.ins.sync_info
    waits = list(si.on_wait) if si and si.on_wait else []
    if len(waits) > 1:
        drain_inst.ins.sync_info = mybir.SyncInfo(
            on_wait=waits[:1], on_update=list(si.on_update or [])
        )
        for w in waits[1:]:
            extra = nc.sync.drain()
            esi = extra.ins.sync_info
            extra.ins.sync_info = mybir.SyncInfo(
                on_wait=[w],
                on_update=list(esi.on_update or []) if esi else [],
            )
    nc.all_engine_barrier()
    assert self.sems is not None
    popped = nc._tile_sem_poison_stack.pop()
    assert popped is self._sem_poison
    nc.clear_and_free_semaphores(list(self.sems.allocated().values()))
    nc.all_engine_barrier()


def _apply_tilefix():
    tile.TileContext._commit_instruction = _commit_instruction
    tile.TileContext._drain_and_barrier = _drain_and_barrier


_apply_tilefix()

# ---------------------------------------------------------------------------
# Problem constants (hardcoded)
# ---------------------------------------------------------------------------
B, S, E, H = 2, 2048, 1024, 16
HC, D = 4, 64              # heads per core, head dim
NCORES = 8
NE = E // 128              # 8  e-chunks (projection contraction)
NK = S // 128              # 16 k-chunks
NQC = S // 512             # 4  512-wide q windows
NM = S // 128              # 16 output row chunks

F32 = mybir.dt.float32
F32R = mybir.dt.float32r
FP16 = mybir.dt.float16
EXP = mybir.ActivationFunctionType.Exp
MULT = mybir.AluOpType.mult


def build(ptbufs=20, xdt=FP16, qkdt=mybir.dt.float32r):
    nc = bass.Bass()
    xqT = nc.dram_tensor("xqT", [E, S], xdt, kind="ExternalInput")
    xkT = nc.dram_tensor("xkT", [E, S], xdt, kind="ExternalInput")
    xvT = nc.dram_tensor("xvT", [E, S], xdt, kind="ExternalInput")
    wqT = nc.dram_tensor("wqT", [E, 256], xdt, kind="ExternalInput")
    wkT = nc.dram_tensor("wkT", [E, 256], xdt, kind="ExternalInput")
    wvT = nc.dram_tensor("wvT", [E, 256], xdt, kind="ExternalInput")
    woT2 = nc.dram_tensor("woT2", [128, 2 * E], FP16, kind="ExternalInput")
    ident = nc.dram_tensor("ident", [128, 128], FP16, kind="ExternalInput")
    out = nc.dram_tensor("out", [S, E], FP16, kind="ExternalOutput")

    with tile.TileContext(nc) as tc, ExitStack() as ctx:
        consts = ctx.enter_context(tc.tile_pool(name="consts", bufs=1))
        wpool = ctx.enter_context(tc.tile_pool(name="w", bufs=1))
        actpool = ctx.enter_context(tc.tile_pool(name="acts", bufs=1))
        xkvpool = ctx.enter_context(tc.tile_pool(name="xkv", bufs=10))
        xqpool = ctx.enter_context(tc.tile_pool(name="xq", bufs=4))

        # preload the exp table before the hot loop
        dummy = consts.tile([1, 8], F32)
        nc.vector.memset(dummy[:], 0.0)
        nc.scalar.activation(dummy[:], dummy[:], EXP)

        wq_sb = wpool.tile([128, NE, 256], xdt)
        wk_sb = wpool.tile([128, NE, 256], xdt)
        wv_sb = wpool.tile([128, NE, 256], xdt)
        wo2_sb = wpool.tile([128, 2, E], FP16)
        id_sb = wpool.tile([128, 128], FP16)

        # per-pair tensors: tile-granular dependency tracking means a read
        # waits on every writer emitted so far, so pair0's scores must not
        # share a tile with pair1's (later-copied) projections
        qT_sb = [actpool.tile([128, S], qkdt, name=f"qT{p}") for p in range(2)]
        kT_sb = [actpool.tile([128, S], qkdt, name=f"kT{p}") for p in range(2)]
        v_sb = actpool.tile([128, NK, HC, 65], FP16)   # [s%128, k, h, V_h|one]
        oT2_sb = actpool.tile([128, 2, S], FP16)       # [(2 heads x d), pair, q]

        # ---- DMA program: ordered by first-use ----
        nc.sync.dma_start(wk_sb[:], wkT.rearrange("(ec p) n -> p ec n", p=128))
        nc.sync.dma_start(wq_sb[:], wqT.rearrange("(ec p) n -> p ec n", p=128))
        xk = []
        for e in range(NE):
            t = xkvpool.tile([128, S], xdt, tag="xc", name=f"xk{e}")
            nc.sync.dma_start(t[:], xkT[e * 128:(e + 1) * 128, :])
            xk.append(t)
        # xq arrives in per-qc column blocks (ONE descriptor each: HWDGE
        # descriptor-gen at ~625ns/DMA would otherwise pace the 364ns slices)
        xqv = xqT.rearrange("(ec p) s -> p ec s", p=128)
        xq = [None] * NQC
        for qc in [0]:
            t = xqpool.tile([128, NE, 512], xdt, tag="xq", name=f"xq{qc}")
            nc.sync.dma_start(t[:], xqv[:, :, qc * 512:(qc + 1) * 512])
            xq[qc] = t
        xv = []
        for e in range(NE):
            t = xkvpool.tile([128, S], xdt, tag="xc", name=f"xv{e}")
            nc.sync.dma_start(t[:], xvT[e * 128:(e + 1) * 128, :])
            xv.append(t)
        # wv rides behind xv (first use ~32us, xv gates the V projections)
        nc.sync.dma_start(wv_sb[:], wvT.rearrange("(ec p) n -> p ec n", p=128))
        for qc in range(1, NQC):
            if qc == 2:
                # wo2/ident ride between xq1 and xq2 (first use ~55us); they
                # must not delay xq1, which gates the qc1 Q-projection
                nc.sync.dma_start(
                    wo2_sb[:], woT2.rearrange("p (two e) -> p two e", two=2))
                nc.sync.dma_start(id_sb[:], ident[0:128, :])
            t = xqpool.tile([128, NE, 512], xdt, tag="xq", name=f"xq{qc}")
            nc.sync.dma_start(t[:], xqv[:, :, qc * 512:(qc + 1) * 512])
            xq[qc] = t

        # ones column for the fused rowsum (PV moving operand col 64)
        nc.vector.memset(v_sb[:, :, :, 64:65], 1.0)

        # ---- steady-state pools (psS first: it coexists with prefix psA) ----
        psS = ctx.enter_context(tc.tile_pool(name="psS", bufs=2, space="PSUM"))
        ppool = ctx.enter_context(tc.tile_pool(name="pT", bufs=ptbufs))
        npool = ctx.enter_context(tc.tile_pool(name="nT", bufs=6))
        rpool = ctx.enter_context(tc.tile_pool(name="rt", bufs=4))
        stpool = ctx.enter_context(tc.tile_pool(name="stage", bufs=4))
        scpool = ctx.enter_context(tc.tile_pool(name="oscr", bufs=2))

        def emit_scores_exp(qc, pair, k):
            qs = slice(qc * 512, (qc + 1) * 512)
            ks = slice(k * 128, (k + 1) * 128)
            ps_s = psS.tile([128, 1024], F32, tag="ss", name=f"ss{qc}_{pair}_{k}")
            nc.tensor.matmul(ps_s[:, 0:512],
                             kT_sb[pair][0:64, ks], qT_sb[pair][0:64, qs],
                             start=True, stop=True)
            nc.tensor.matmul(ps_s[:, 512:1024],
                             kT_sb[pair][64:128, ks], qT_sb[pair][64:128, qs],
                             start=True, stop=True)
            pT = ppool.tile([128, 1024], FP16, tag="pt", name=f"pt{qc}_{pair}_{k}")
            nc.scalar.activation(pT[:], ps_s[:], EXP, scale=0.125)
            return pT

        def emit_pv(pair, k, pT, ovv, qts=(0, 1, 2, 3)):
            # ovv: ps_o viewed as [128, qt, h2, 128]; O in cols 0:64, sum col
            # 64. qt-major so qt 0-1 fill PSUM bank0 and qt 2-3 bank1 (the
            # last window is drained per qt-half). PSUM zero regions are 2KB:
            # start=True pending-zeroes the WHOLE bank, so only the first
            # region per bank may start the group; the other regions' k==0
            # writes land on pending-zero bytes and write-through (hardware
            # lazy-zero semantics).
            for qt in qts:
                for h2 in range(2):
                    h = pair * 2 + h2
                    nc.tensor.matmul(
                        ovv[:, qt, h2, 0:65],
                        pT[:, h2 * 512 + qt * 128:h2 * 512 + (qt + 1) * 128],
                        v_sb[:, k, h, 0:65],
                        start=(k == 0 and h2 == 0 and qt in (0, 2)),
                        stop=(k == NK - 1),
                        skip_group_check=True,
                    )

        def emit_vproj(k):
            t = psM.tile([128, 512], F32, tag="mi", name=f"vp{k}")
            for e in range(NE):
                nc.tensor.matmul(
                    t[:, 0:256],
                    xv[e][:, k * 128:(k + 1) * 128],
                    wv_sb[:, e, :],
                    start=(e == 0), stop=(e == NE - 1),
                )
            nc.vector.tensor_copy(
                v_sb[:, k, :, 0:64],
                t[:, 0:256].rearrange("p (h c) -> p h c", h=HC))

        def emit_qproj(qc, nch):
            t = psM.tile([128, 512], F32, tag="mi", name=f"qp{qc}_{nch}")
            for e in range(NE):
                nc.tensor.matmul(
                    t[:],
                    wq_sb[:, e, nch * 128:(nch + 1) * 128],
                    xq[qc][:, e, :],
                    start=(e == 0), stop=(e == NE - 1),
                )
            nc.vector.tensor_copy(qT_sb[nch][:, qc * 512:(qc + 1) * 512], t[:])

        def emit_norm(qc, pair, ovv, qts=(0, 1, 2, 3), split=False):
            # Mid-stream: evacuate ps_o to SBUF in ONE copy so the next
            # window's PV (which reuses the psOV ring slot) waits ~1.2us
            # instead of ~3us of piecemeal norm reads.
            if not split:
                osc = scpool.tile([128, 1024], F32, tag="os",
                                  name=f"os{qc}_{pair}")
                nc.vector.tensor_copy(osc[:], ovv.rearrange(
                    "p q h c -> p (q h c)"))
                ovv = osc.rearrange("p (q h c) -> p q h c", q=4, h=2)
            # reciprocal of the fused rowsums (col 64 of each 128-col region)
            n = len(qts)
            rt = rpool.tile([128, 2 * n], F32, tag="rt",
                            name=f"rt{qc}_{pair}_{qts[0]}")
            nc.vector.reciprocal(
                rt[:].rearrange("p (q h) -> p q h", q=n),
                ovv[:, qts[0]:qts[0] + n, :, 64])
            nTs = {}
            for i, qt in enumerate(qts):
                nT = npool.tile([128, 128], FP16, tag="nt",
                                name=f"nt{qc}_{pair}_{qt}")
                for h2 in range(2):
                    if split and qt >= 2:
                        # tail only: whole tiles on the idle Act engine
                        # (mixed engines per tile serialize on WAW sems)
                        nc.scalar.mul(nT[:, h2 * 64:(h2 + 1) * 64],
                                      ovv[:, qt, h2, 0:64],
                                      rt[:, 2 * i + h2:2 * i + h2 + 1])
                    else:
                        nc.vector.tensor_scalar(
                            nT[:, h2 * 64:(h2 + 1) * 64],
                            ovv[:, qt, h2, 0:64],
                            rt[:, 2 * i + h2:2 * i + h2 + 1],
                            None, MULT)
                nTs[qt] = nT
            return nTs

        def emit_transpose(qc, pair, qt, nT, pool=None):
            # pool=psS in the tail: scores are done, its slots are idle, and
            # keeping transposes out of the psM ring stops them serializing
            # behind out-proj tiles awaiting their stage copies.
            pool = pool or psM
            tag = {id(psS): "ss", id(psOV): "ov"}.get(id(pool), "mi")
            psT = pool.tile([128, 128], FP16, tag=tag, name=f"tp{qc}_{pair}_{qt}")
            nc.tensor.transpose(psT[:], nT[:], id_sb[:])
            nc.vector.tensor_copy(
                oT2_sb[:, pair, qc * 512 + qt * 128:qc * 512 + (qt + 1) * 128],
                psT[:])

        def emit_outproj(m, split=False, pool=None):
            pool = pool or psM
            tag = {id(psS): "ss", id(psOV): "ov"}.get(id(pool), "mi")
            stage = stpool.tile([128, E], FP16, tag="st", name=f"st{m}")
            for j in range(2):
                t = pool.tile([128, 512], F32, tag=tag, name=f"op{m}_{j}")
                for pair in range(2):
                    nc.tensor.matmul(
                        t[:],
                        oT2_sb[:, pair, m * 128:(m + 1) * 128],
                        wo2_sb[:, pair, j * 512:(j + 1) * 512],
                        start=(pair == 0), stop=(pair == 1),
                    )
                if split and j == 1:
                    nc.scalar.copy(stage[:, 512:1024], t[:])
                else:
                    nc.vector.tensor_copy(stage[:, j * 512:(j + 1) * 512], t[:])
            if split:
                nc.sync.dma_start(out[m * 128:(m + 1) * 128, :], stage[:])
            else:
                nc.gpsimd.dma_start(out[m * 128:(m + 1) * 128, :], stage[:])

        # ---- prefix: K/Q projections, nch-split so pair0 lands fast ----
        # psA (4 banks) coexists with psS (4 banks); pair0's K tiles project
        # and copy out first, Q-qc0 goes through psS slots, and the first
        # scores fire while pair1's K projection finishes in the background.
        pts = {}
        cur = []
        vdone = 0
        with tc.tile_pool(name="psA", bufs=4, space="PSUM") as psA:
            kt0 = [psA.tile([128, 512], F32, tag="mm", name=f"pk0_{m}")
                   for m in range(4)]
            kt1 = [psA.tile([128, 512], F32, tag="mm", name=f"pk1_{m}")
                   for m in range(4)]
            for e in range(NE):
                for m in range(4):
                    nc.tensor.matmul(
                        kt0[m][:], wk_sb[:, e, 0:128],
                        xk[e][:, m * 512:(m + 1) * 512],
                        start=(e == 0), stop=(e == NE - 1))
            for m in range(4):
                nc.vector.tensor_copy(
                    kT_sb[0][:, m * 512:(m + 1) * 512], kt0[m][:])
            # Q-qc0 pair0 chases the xq0 DMAs immediately after K-pair0 (the
            # PE stream is in-order, so nothing may sit between them); all of
            # pair1's projection work weaves into the first scores' shadows.
            tq0 = psS.tile([128, 512], F32, tag="ss", name="pq0_0")
            # tq1 must NOT take a psS ring slot: its reader (the qT1 copy)
            # lands after the early scores, which would deadlock their
            # tile allocations. The psA ring's kt0[0] slot frees early.
            tq1 = psA.tile([128, 512], F32, tag="mm", name="pq0_1")
            for e in range(NE):
                nc.tensor.matmul(
                    tq0[:], wq_sb[:, e, 0:128], xq[0][:, e, :],
                    start=(e == 0), stop=(e == NE - 1))
            nc.vector.tensor_copy(qT_sb[0][:, 0:512], tq0[:])
            for k in range(6):
                cur.append(emit_scores_exp(0, 0, k))
                if k < 4:     # K-pair1, two e-chunks per score shadow
                    for e in (2 * k, 2 * k + 1):
                        for m in range(4):
                            nc.tensor.matmul(
                                kt1[m][:], wk_sb[:, e, 128:256],
                                xk[e][:, m * 512:(m + 1) * 512],
                                start=(e == 0), stop=(e == NE - 1))
                elif k < 6:   # then Q-pair1, four e-chunks per shadow
                    for e in range(4 * (k - 4), 4 * (k - 3)):
                        nc.tensor.matmul(
                            tq1[:], wq_sb[:, e, 128:256], xq[0][:, e, :],
                            start=(e == 0), stop=(e == NE - 1))
            for m in range(4):
                nc.vector.tensor_copy(
                    kT_sb[1][:, m * 512:(m + 1) * 512], kt1[m][:])
            for k in range(6, 8):
                cur.append(emit_scores_exp(0, 0, k))
                if k == 6:
                    nc.vector.tensor_copy(qT_sb[1][:, 0:512], tq1[:])
        psOV = ctx.enter_context(tc.tile_pool(name="psOV", bufs=1, space="PSUM"))
        psM = ctx.enter_context(tc.tile_pool(name="psM", bufs=2, space="PSUM"))

        for k in range(8, NK):
            cur.append(emit_scores_exp(0, 0, k))
            if k >= 9 and vdone < 5:   # xv DMAs have landed by these shadows
                emit_vproj(vdone)
                vdone += 1
            elif k == 14:
                emit_qproj(1, 0)
            elif k == 15:
                emit_qproj(1, 1)
        pts[(0, 0)] = cur

        seq = [(qc, pair) for qc in range(NQC) for pair in range(2)]
        trans_pending = None   # (qc, pair, nTs) awaiting transpose weave
        for i, (qc, pair) in enumerate(seq):
            nxt = seq[i + 1] if i + 1 < len(seq) else None
            ps_o = psOV.tile([128, 1024], F32, tag="ov", name=f"ov{qc}_{pair}")
            ovv = ps_o.rearrange("p (q h c) -> p q h c", q=4, h=2)
            cur_pts = pts.pop((qc, pair))
            nxt_pts = [] if nxt else None
            op_ms = list(range((qc - 1) * 4, qc * 4)) if (pair == 0 and qc >= 1) else []
            last_qts = (0, 1, 2, 3)
            for k in range(NK):
                if nxt:
                    nxt_pts.append(emit_scores_exp(nxt[0], nxt[1], k))
                if trans_pending and k in (2, 4, 6, 8):
                    tqc, tpair, tnTs = trans_pending
                    emit_transpose(tqc, tpair, k // 2 - 1, tnTs[k // 2 - 1])
                    if k == 8:
                        trans_pending = None
                if vdone < NK:  # remaining V-proj tiles, just-in-time
                    emit_vproj(vdone)
                    vdone += 1
                emit_pv(pair, k, cur_pts[k], ovv, qts=last_qts)
                if op_ms and k % 4 == 2:
                    emit_outproj(op_ms[k // 4])
                if pair == 0 and 1 <= qc < NQC - 1 and k in (8, 12):
                    # Q projection for the next qc, before its scores appear
                    emit_qproj(qc + 1, (k - 8) // 4)
            if nxt:
                pts[nxt] = nxt_pts
                nTs = emit_norm(qc, pair, ovv)
                trans_pending = (qc, pair, nTs)
            else:
                # tail drain: norm (split across DVE + the idle Act engine),
                # then per-q-tile transpose + out-proj chains distributed
                # over all three PSUM rings so they pipeline
                mb = (NQC - 1) * 4
                nTs = emit_norm(qc, pair, ovv, split=True)
                emit_transpose(qc, pair, 0, nTs[0], pool=psS)
                emit_transpose(qc, pair, 1, nTs[1], pool=psS)
                emit_transpose(qc, pair, 2, nTs[2], pool=psOV)
                emit_transpose(qc, pair, 3, nTs[3], pool=psOV)
                emit_outproj(mb + 0, split=True, pool=psM)
                emit_outproj(mb + 1, split=True, pool=psS)
                emit_outproj(mb + 2, split=True, pool=psM)
                emit_outproj(mb + 3, split=True, pool=psS)

    return nc


_NC_CACHE = {}


def _get_nc():
    if "nc" not in _NC_CACHE:
        _NC_CACHE["nc"] = build()
    return _NC_CACHE["nc"]


def _shard_inputs(query, key, value, Wq, Wk, Wv, Wo):
    """Host-side sharding + layout prep: core c = (batch c//4, head-group c%4)."""
    f16 = np.float16
    xT = []
    for b in range(B):
        xT.append((
            np.ascontiguousarray(query[b].T).astype(f16),
            np.ascontiguousarray(key[b].T).astype(f16),
            np.ascontiguousarray(value[b].T).astype(f16),
        ))
    wT = []
    for g in range(4):
        gc = slice(g * 256, (g + 1) * 256)
        wo_g = Wo[:, gc].T.astype(f16)            # [256, E]
        woT2 = np.ascontiguousarray(
            wo_g.reshape(2, 128, E).transpose(1, 0, 2).reshape(128, 2 * E))
        wT.append((
            np.ascontiguousarray(Wq[gc].T).astype(f16),
            np.ascontiguousarray(Wk[gc].T).astype(f16),
            np.ascontiguousarray(Wv[gc].T).astype(f16),
            woT2,
        ))
    ident = np.eye(128, dtype=f16)
    in_maps = []
    for c in range(NCORES):
        b, g = c // 4, c % 4
        qT, kT, vT = xT[b]
        wq, wk, wv, wo2 = wT[g]
        in_maps.append({
            "xqT": qT, "xkT": kT, "xvT": vT,
            "wqT": wq, "wkT": wk, "wvT": wv, "woT2": wo2,
            "ident": ident,
        })
    return in_maps


def kernel(query, key, value, Wq, Wk, Wv, Wo):
    query = np.asarray(query, dtype=np.float32)
    key = np.asarray(key, dtype=np.float32)
    value = np.asarray(value, dtype=np.float32)
    Wq = np.asarray(Wq, dtype=np.float32)
    Wk = np.asarray(Wk, dtype=np.float32)
    Wv = np.asarray(Wv, dtype=np.float32)
    Wo = np.asarray(Wo, dtype=np.float32)

    nc = _get_nc()
    in_maps = _shard_inputs(query, key, value, Wq, Wk, Wv, Wo)
    res = run_bass_kernel_spmd(nc, in_maps, core_ids=list(range(NCORES)))

    out = np.zeros((B, S, E), dtype=np.float32)
    for c in range(NCORES):
        out[c // 4] += res.results[c]["out"].astype(np.float32)
    return out


# revision 71
# speedup vs baseline: 1.0126x; 1.0017x over previous
"""Trainium2 Bass kernel for nn_MultiHeadAttention (B=2, S=2048, E=1024, H=16).

Sharding: 8 NeuronCores = data-parallel over the 2 batches x tensor-parallel
over the 16 heads in 4 groups of 4 heads (Wq/Wk/Wv split column-wise, Wo
row-wise).  Each core computes a full-[S, E] partial of its batch's output;
the host sums the 4 head-group partials per batch.

Per-core device algorithm (S.T orientation feeds a flipped P@V):
  Q.T/K.T[n, s] = (wT chunk).T @ xT chunk       e/m-outer projections
  S.T_h[k, q]   = (K_h.T chunk).T @ Q_h.T       row-packed head pairs (d=64)
  P.T           = exp(S.T / 8)                  fp16, one ACT op per (k, pair)
  O[q, d|sum]   = (P.T chunk).T @ [V_h | 1]     FLIPPED: P.T [128k,128q] is the
                                                stationary, [V|ones] [128k,65]
                                                moving -> 65-col outputs, with
                                                the softmax denominator landing
                                                in column 64 (per-partition!)
  O_n           = O * recip(col 64)             one DVE tensor_scalar per tile
  O.T           = PE-transpose(O_n)             53ns/tile, restores [d, q]
  out[m, :]     = sum_pair (oT2 chunk).T @ woT2 contract-128 output projection

The flip + contract-128 out-proj cut PE matmul time ~27%; exp on the
Activation engine (~133us) becomes the critical resource, so the instruction
stream is interleaved to keep it saturated: each (qc, pair) "window" weaves
the NEXT window's scores+exp with THIS window's PV, plus V/Q-projection,
out-projection, and transpose filler work sized to the Act-engine pace.

dtypes: HBM traffic fp16 (in+out); scores fp32r; P/V/O/Wo fp16; accum fp32.
"""

import numpy as np
from contextlib import ExitStack

import ml_dtypes

import concourse.bass as bass
import concourse.mybir as mybir
import concourse.tile as tile
from concourse.tile import ScopedClock
from concourse.bass_utils import run_bass_kernel_spmd

# ---------------------------------------------------------------------------
# Workarounds for the walrus build on this stack, which rejects more than ONE
# semaphore wait per instruction ("Too many sync wait commands").
# ---------------------------------------------------------------------------
_orig_commit_instruction = tile.TileContext._commit_instruction


def _commit_instruction(self, inst, lazy_reg_writes=True):
    si = getattr(inst, "sync_info", None)
    if si is not None and si.on_wait and len(si.on_wait) > 1:
        waits = list(si.on_wait)
        for w in waits[:-1]:
            nop = mybir.InstNoOp(
                name=self.nc.get_next_instruction_name(),
                ins=[], outs=[], engine=inst.engine,
            )
            nop.bass_nofuse = True
            nop.sync_info = mybir.SyncInfo(on_wait=[w], on_update=[])
            _orig_commit_instruction(self, nop, lazy_reg_writes=False)
        inst.sync_info = mybir.SyncInfo(
            on_wait=[waits[-1]], on_update=list(si.on_update or [])
        )
    return _orig_commit_instruction(self, inst, lazy_reg_writes)


def _drain_and_barrier(self, tick_clock, wait_clock):
    nc = self.nc
    drain_inst = nc.sync.drain()
    wait_clock.add_sem_waits(
        drain_inst.ins, ScopedClock({None: tick_clock.global_clock})
    )
    si = drain_inst.ins.sync_info
    waits = list(si.on_wait) if si and si.on_wait else []
    if len(waits) > 1:
        drain_inst.ins.sync_info = mybir.SyncInfo(
            on_wait=waits[:1], on_update=list(si.on_update or [])
        )
        for w in waits[1:]:
            extra = nc.sync.drain()
            esi = extra.ins.sync_info
            extra.ins.sync_info = mybir.SyncInfo(
                on_wait=[w],
                on_update=list(esi.on_update or []) if esi else [],
            )
    nc.all_engine_barrier()
    assert self.sems is not None
    popped = nc._tile_sem_poison_stack.pop()
    assert popped is self._sem_poison
    nc.clear_and_free_semaphores(list(self.sems.allocated().values()))
    nc.all_engine_barrier()


def _apply_tilefix():
    tile.TileContext._commit_instruction = _commit_instruction
    tile.TileContext._drain_and_barrier = _drain_and_barrier


_apply_tilefix()

# ---------------------------------------------------------------------------
# Problem constants (hardcoded)
# ---------------------------------------------------------------------------
B, S, E, H = 2, 2048, 1024, 16
HC, D = 4, 64              # heads per core, head dim
NCORES = 8
NE = E // 128              # 8  e-chunks (projection contraction)
NK = S // 128              # 16 k-chunks
NQC = S // 512             # 4  512-wide q windows
NM = S // 128              # 16 output row chunks

F32 = mybir.dt.float32
F32R = mybir.dt.float32r
FP16 = mybir.dt.float16
EXP = mybir.ActivationFunctionType.Exp
MULT = mybir.AluOpType.mult


def build(ptbufs=20, xdt=FP16, qkdt=mybir.dt.float32r):
    nc = bass.Bass()
    xqT = nc.dram_tensor("xqT", [E, S], xdt, kind="ExternalInput")
    xkT = nc.dram_tensor("xkT", [E, S], xdt, kind="ExternalInput")
    xvT = nc.dram_tensor("xvT", [E, S], xdt, kind="ExternalInput")
    wqT = nc.dram_tensor("wqT", [E, 256], xdt, kind="ExternalInput")
    wkT = nc.dram_tensor("wkT", [E, 256], xdt, kind="ExternalInput")
    wvT = nc.dram_tensor("wvT", [E, 256], xdt, kind="ExternalInput")
    woT2 = nc.dram_tensor("woT2", [128, 2 * E], FP16, kind="ExternalInput")
    ident = nc.dram_tensor("ident", [128, 128], FP16, kind="ExternalInput")
    out = nc.dram_tensor("out", [S, E], FP16, kind="ExternalOutput")

    with tile.TileContext(nc) as tc, ExitStack() as ctx:
        consts = ctx.enter_context(tc.tile_pool(name="consts", bufs=1))
        wpool = ctx.enter_context(tc.tile_pool(name="w", bufs=1))
        actpool = ctx.enter_context(tc.tile_pool(name="acts", bufs=1))
        xkvpool = ctx.enter_context(tc.tile_pool(name="xkv", bufs=10))
        xqpool = ctx.enter_context(tc.tile_pool(name="xq", bufs=4))

        # preload the exp table before the hot loop
        dummy = consts.tile([1, 8], F32)
        nc.vector.memset(dummy[:], 0.0)
        nc.scalar.activation(dummy[:], dummy[:], EXP)

        wq_sb = wpool.tile([128, NE, 256], xdt)
        wk_sb = wpool.tile([128, NE, 256], xdt)
        wv_sb = wpool.tile([128, NE, 256], xdt)
        wo2_sb = wpool.tile([128, 2, E], FP16)
        id_sb = wpool.tile([128, 128], FP16)

        # per-pair tensors: tile-granular dependency tracking means a read
        # waits on every writer emitted so far, so pair0's scores must not
        # share a tile with pair1's (later-copied) projections
        qT_sb = [actpool.tile([128, S], qkdt, name=f"qT{p}") for p in range(2)]
        kT_sb = [actpool.tile([128, S], qkdt, name=f"kT{p}") for p in range(2)]
        v_sb = actpool.tile([128, NK, HC, 65], FP16)   # [s%128, k, h, V_h|one]
        oT2_sb = actpool.tile([128, 2, S], FP16)       # [(2 heads x d), pair, q]

        # ---- DMA program: ordered by first-use ----
        nc.sync.dma_start(wk_sb[:], wkT.rearrange("(ec p) n -> p ec n", p=128))
        nc.sync.dma_start(wq_sb[:], wqT.rearrange("(ec p) n -> p ec n", p=128))
        xk = []
        for e in range(NE):
            t = xkvpool.tile([128, S], xdt, tag="xc", name=f"xk{e}")
            nc.sync.dma_start(t[:], xkT[e * 128:(e + 1) * 128, :])
            xk.append(t)
        # xq arrives in per-qc column blocks (ONE descriptor each: HWDGE
        # descriptor-gen at ~625ns/DMA would otherwise pace the 364ns slices)
        xqv = xqT.rearrange("(ec p) s -> p ec s", p=128)
        xq = [None] * NQC
        for qc in [0]:
            t = xqpool.tile([128, NE, 512], xdt, tag="xq", name=f"xq{qc}")
            nc.sync.dma_start(t[:], xqv[:, :, qc * 512:(qc + 1) * 512])
            xq[qc] = t
        xv = []
        for e in range(NE):
            t = xkvpool.tile([128, S], xdt, tag="xc", name=f"xv{e}")
            nc.sync.dma_start(t[:], xvT[e * 128:(e + 1) * 128, :])
            xv.append(t)
        # wv rides behind xv (first use ~32us, xv gates the V projections)
        nc.sync.dma_start(wv_sb[:], wvT.rearrange("(ec p) n -> p ec n", p=128))
        for qc in range(1, NQC):
            if qc == 2:
                # wo2/ident ride between xq1 and xq2 (first use ~55us); they
                # must not delay xq1, which gates the qc1 Q-projection
                nc.sync.dma_start(
                    wo2_sb[:], woT2.rearrange("p (two e) -> p two e", two=2))
                nc.sync.dma_start(id_sb[:], ident[0:128, :])
            t = xqpool.tile([128, NE, 512], xdt, tag="xq", name=f"xq{qc}")
            nc.sync.dma_start(t[:], xqv[:, :, qc * 512:(qc + 1) * 512])
            xq[qc] = t

        # ones column for the fused rowsum (PV moving operand col 64)
        nc.vector.memset(v_sb[:, :, :, 64:65], 1.0)

        # ---- steady-state pools (psS first: it coexists with prefix psA) ----
        psS = ctx.enter_context(tc.tile_pool(name="psS", bufs=2, space="PSUM"))
        ppool = ctx.enter_context(tc.tile_pool(name="pT", bufs=ptbufs))
        npool = ctx.enter_context(tc.tile_pool(name="nT", bufs=6))
        rpool = ctx.enter_context(tc.tile_pool(name="rt", bufs=4))
        stpool = ctx.enter_context(tc.tile_pool(name="stage", bufs=4))
        scpool = ctx.enter_context(tc.tile_pool(name="oscr", bufs=2))

        def emit_scores_exp(qc, pair, k):
            qs = slice(qc * 512, (qc + 1) * 512)
            ks = slice(k * 128, (k + 1) * 128)
            ps_s = psS.tile([128, 1024], F32, tag="ss", name=f"ss{qc}_{pair}_{k}")
            nc.tensor.matmul(ps_s[:, 0:512],
                             kT_sb[pair][0:64, ks], qT_sb[pair][0:64, qs],
                             start=True, stop=True)
            nc.tensor.matmul(ps_s[:, 512:1024],
                             kT_sb[pair][64:128, ks], qT_sb[pair][64:128, qs],
                             start=True, stop=True)
            pT = ppool.tile([128, 1024], FP16, tag="pt", name=f"pt{qc}_{pair}_{k}")
            nc.scalar.activation(pT[:], ps_s[:], EXP, scale=0.125)
            return pT

        def emit_pv(pair, k, pT, ovv, qts=(0, 1, 2, 3)):
            # ovv: ps_o viewed as [128, qt, h2, 128]; O in cols 0:64, sum col
            # 64. qt-major so qt 0-1 fill PSUM bank0 and qt 2-3 bank1 (the
            # last window is drained per qt-half). PSUM zero regions are 2KB:
            # start=True pending-zeroes the WHOLE bank, so only the first
            # region per bank may start the group; the other regions' k==0
            # writes land on pending-zero bytes and write-through (hardware
            # lazy-zero semantics).
            for qt in qts:
                for h2 in range(2):
                    h = pair * 2 + h2
                    nc.tensor.matmul(
                        ovv[:, qt, h2, 0:65],
                        pT[:, h2 * 512 + qt * 128:h2 * 512 + (qt + 1) * 128],
                        v_sb[:, k, h, 0:65],
                        start=(k == 0 and h2 == 0 and qt in (0, 2)),
                        stop=(k == NK - 1),
                        skip_group_check=True,
                    )

        def emit_vproj(k):
            t = psM.tile([128, 512], F32, tag="mi", name=f"vp{k}")
            for e in range(NE):
                nc.tensor.matmul(
                    t[:, 0:256],
                    xv[e][:, k * 128:(k + 1) * 128],
                    wv_sb[:, e, :],
                    start=(e == 0), stop=(e == NE - 1),
                )
            nc.vector.tensor_copy(
                v_sb[:, k, :, 0:64],
                t[:, 0:256].rearrange("p (h c) -> p h c", h=HC))

        def emit_qproj(qc, nch):
            t = psM.tile([128, 512], F32, tag="mi", name=f"qp{qc}_{nch}")
            for e in range(NE):
                nc.tensor.matmul(
                    t[:],
                    wq_sb[:, e, nch * 128:(nch + 1) * 128],
                    xq[qc][:, e, :],
                    start=(e == 0), stop=(e == NE - 1),
                )
            nc.vector.tensor_copy(qT_sb[nch][:, qc * 512:(qc + 1) * 512], t[:])

        def emit_norm(qc, pair, ovv, qts=(0, 1, 2, 3), split=False):
            # Mid-stream: evacuate ps_o to SBUF in ONE copy so the next
            # window's PV (which reuses the psOV ring slot) waits ~1.2us
            # instead of ~3us of piecemeal norm reads.
            if True:
                osc = scpool.tile([128, 1024], F32, tag="os",
                                  name=f"os{qc}_{pair}")
                nc.vector.tensor_copy(osc[:], ovv.rearrange(
                    "p q h c -> p (q h c)"))
                ovv = osc.rearrange("p (q h c) -> p q h c", q=4, h=2)
            # reciprocal of the fused rowsums (col 64 of each 128-col region)
            n = len(qts)
            rt = rpool.tile([128, 2 * n], F32, tag="rt",
                            name=f"rt{qc}_{pair}_{qts[0]}")
            nc.vector.reciprocal(
                rt[:].rearrange("p (q h) -> p q h", q=n),
                ovv[:, qts[0]:qts[0] + n, :, 64])
            nTs = {}
            for i, qt in enumerate(qts):
                nT = npool.tile([128, 128], FP16, tag="nt",
                                name=f"nt{qc}_{pair}_{qt}")
                for h2 in range(2):
                    if split and qt >= 2:
                        # tail only: whole tiles on the idle Act engine
                        # (mixed engines per tile serialize on WAW sems)
                        nc.scalar.mul(nT[:, h2 * 64:(h2 + 1) * 64],
                                      ovv[:, qt, h2, 0:64],
                                      rt[:, 2 * i + h2:2 * i + h2 + 1])
                    else:
                        nc.vector.tensor_scalar(
                            nT[:, h2 * 64:(h2 + 1) * 64],
                            ovv[:, qt, h2, 0:64],
                            rt[:, 2 * i + h2:2 * i + h2 + 1],
                            None, MULT)
                nTs[qt] = nT
            return nTs

        def emit_transpose(qc, pair, qt, nT, pool=None):
            # pool=psS in the tail: scores are done, its slots are idle, and
            # keeping transposes out of the psM ring stops them serializing
            # behind out-proj tiles awaiting their stage copies.
            pool = pool or psM
            tag = {id(psS): "ss", id(psOV): "ov"}.get(id(pool), "mi")
            psT = pool.tile([128, 128], FP16, tag=tag, name=f"tp{qc}_{pair}_{qt}")
            nc.tensor.transpose(psT[:], nT[:], id_sb[:])
            nc.vector.tensor_copy(
                oT2_sb[:, pair, qc * 512 + qt * 128:qc * 512 + (qt + 1) * 128],
                psT[:])

        def emit_outproj(m, split=False, pool=None):
            pool = pool or psM
            tag = {id(psS): "ss", id(psOV): "ov"}.get(id(pool), "mi")
            stage = stpool.tile([128, E], FP16, tag="st", name=f"st{m}")
            for j in range(2):
                t = pool.tile([128, 512], F32, tag=tag, name=f"op{m}_{j}")
                for pair in range(2):
                    nc.tensor.matmul(
                        t[:],
                        oT2_sb[:, pair, m * 128:(m + 1) * 128],
                        wo2_sb[:, pair, j * 512:(j + 1) * 512],
                        start=(pair == 0), stop=(pair == 1),
                    )
                if split and j == 1:
                    nc.scalar.copy(stage[:, 512:1024], t[:])
                else:
                    nc.vector.tensor_copy(stage[:, j * 512:(j + 1) * 512], t[:])
            if split:
                nc.sync.dma_start(out[m * 128:(m + 1) * 128, :], stage[:])
            else:
                nc.gpsimd.dma_start(out[m * 128:(m + 1) * 128, :], stage[:])

        # ---- prefix: K/Q projections, nch-split so pair0 lands fast ----
        # psA (4 banks) coexists with psS (4 banks); pair0's K tiles project
        # and copy out first, Q-qc0 goes through psS slots, and the first
        # scores fire while pair1's K projection finishes in the background.
        pts = {}
        cur = []
        vdone = 0
        with tc.tile_pool(name="psA", bufs=4, space="PSUM") as psA:
            kt0 = [psA.tile([128, 512], F32, tag="mm", name=f"pk0_{m}")
                   for m in range(4)]
            kt1 = [psA.tile([128, 512], F32, tag="mm", name=f"pk1_{m}")
                   for m in range(4)]
            for e in range(NE):
                for m in range(4):
                    nc.tensor.matmul(
                        kt0[m][:], wk_sb[:, e, 0:128],
                        xk[e][:, m * 512:(m + 1) * 512],
                        start=(e == 0), stop=(e == NE - 1))
            for m in range(4):
                nc.vector.tensor_copy(
                    kT_sb[0][:, m * 512:(m + 1) * 512], kt0[m][:])
            # Q-qc0 pair0 chases the xq0 DMAs immediately after K-pair0 (the
            # PE stream is in-order, so nothing may sit between them); all of
            # pair1's projection work weaves into the first scores' shadows.
            tq0 = psS.tile([128, 512], F32, tag="ss", name="pq0_0")
            # tq1 must NOT take a psS ring slot: its reader (the qT1 copy)
            # lands after the early scores, which would deadlock their
            # tile allocations. The psA ring's kt0[0] slot frees early.
            tq1 = psA.tile([128, 512], F32, tag="mm", name="pq0_1")
            for e in range(NE):
                nc.tensor.matmul(
                    tq0[:], wq_sb[:, e, 0:128], xq[0][:, e, :],
                    start=(e == 0), stop=(e == NE - 1))
            nc.vector.tensor_copy(qT_sb[0][:, 0:512], tq0[:])
            for k in range(6):
                cur.append(emit_scores_exp(0, 0, k))
                if k < 4:     # K-pair1, two e-chunks per score shadow
                    for e in (2 * k, 2 * k + 1):
                        for m in range(4):
                            nc.tensor.matmul(
                                kt1[m][:], wk_sb[:, e, 128:256],
                                xk[e][:, m * 512:(m + 1) * 512],
                                start=(e == 0), stop=(e == NE - 1))
                elif k < 6:   # then Q-pair1, four e-chunks per shadow
                    for e in range(4 * (k - 4), 4 * (k - 3)):
                        nc.tensor.matmul(
                            tq1[:], wq_sb[:, e, 128:256], xq[0][:, e, :],
                            start=(e == 0), stop=(e == NE - 1))
            for m in range(4):
                nc.vector.tensor_copy(
                    kT_sb[1][:, m * 512:(m + 1) * 512], kt1[m][:])
            for k in range(6, 8):
                cur.append(emit_scores_exp(0, 0, k))
                if k == 6:
                    nc.vector.tensor_copy(qT_sb[1][:, 0:512], tq1[:])
        psOV = ctx.enter_context(tc.tile_pool(name="psOV", bufs=1, space="PSUM"))
        psM = ctx.enter_context(tc.tile_pool(name="psM", bufs=2, space="PSUM"))

        for k in range(8, NK):
            cur.append(emit_scores_exp(0, 0, k))
            if k >= 9 and vdone < 5:   # xv DMAs have landed by these shadows
                emit_vproj(vdone)
                vdone += 1
            elif k == 14:
                emit_qproj(1, 0)
            elif k == 15:
                emit_qproj(1, 1)
        pts[(0, 0)] = cur

        seq = [(qc, pair) for qc in range(NQC) for pair in range(2)]
        trans_pending = None   # (qc, pair, nTs) awaiting transpose weave
        for i, (qc, pair) in enumerate(seq):
            nxt = seq[i + 1] if i + 1 < len(seq) else None
            ps_o = psOV.tile([128, 1024], F32, tag="ov", name=f"ov{qc}_{pair}")
            ovv = ps_o.rearrange("p (q h c) -> p q h c", q=4, h=2)
            cur_pts = pts.pop((qc, pair))
            nxt_pts = [] if nxt else None
            op_ms = list(range((qc - 1) * 4, qc * 4)) if (pair == 0 and qc >= 1) else []
            last_qts = (0, 1, 2, 3)
            for k in range(NK):
                if nxt:
                    nxt_pts.append(emit_scores_exp(nxt[0], nxt[1], k))
                if trans_pending and k in (2, 4, 6, 8):
                    tqc, tpair, tnTs = trans_pending
                    emit_transpose(tqc, tpair, k // 2 - 1, tnTs[k // 2 - 1])
                    if k == 8:
                        trans_pending = None
                if vdone < NK:  # remaining V-proj tiles, just-in-time
                    emit_vproj(vdone)
                    vdone += 1
                emit_pv(pair, k, cur_pts[k], ovv, qts=last_qts)
                if op_ms and k % 4 == 2:
                    emit_outproj(op_ms[k // 4])
                if pair == 0 and 1 <= qc < NQC - 1 and k in (8, 12):
                    # Q projection for the next qc, before its scores appear
                    emit_qproj(qc + 1, (k - 8) // 4)
            if nxt:
                pts[nxt] = nxt_pts
                nTs = emit_norm(qc, pair, ovv)
                trans_pending = (qc, pair, nTs)
            else:
                # tail drain: norm (split across DVE + the idle Act engine),
                # then per-q-tile transpose + out-proj chains distributed
                # over all three PSUM rings so they pipeline
                mb = (NQC - 1) * 4
                nTs = emit_norm(qc, pair, ovv, split=True)
                emit_transpose(qc, pair, 0, nTs[0], pool=psS)
                emit_transpose(qc, pair, 1, nTs[1], pool=psS)
                emit_transpose(qc, pair, 2, nTs[2], pool=psOV)
                emit_transpose(qc, pair, 3, nTs[3], pool=psOV)
                emit_outproj(mb + 0, split=True, pool=psM)
                emit_outproj(mb + 1, split=True, pool=psS)
                emit_outproj(mb + 2, split=True, pool=psM)
                emit_outproj(mb + 3, split=True, pool=psS)

    return nc


_NC_CACHE = {}


def _get_nc():
    if "nc" not in _NC_CACHE:
        _NC_CACHE["nc"] = build()
    return _NC_CACHE["nc"]


def _shard_inputs(query, key, value, Wq, Wk, Wv, Wo):
    """Host-side sharding + layout prep: core c = (batch c//4, head-group c%4)."""
    f16 = np.float16
    xT = []
    for b in range(B):
        xT.append((
            np.ascontiguousarray(query[b].T).astype(f16),
            np.ascontiguousarray(key[b].T).astype(f16),
            np.ascontiguousarray(value[b].T).astype(f16),
        ))
    wT = []
    for g in range(4):
        gc = slice(g * 256, (g + 1) * 256)
        wo_g = Wo[:, gc].T.astype(f16)            # [256, E]
        woT2 = np.ascontiguousarray(
            wo_g.reshape(2, 128, E).transpose(1, 0, 2).reshape(128, 2 * E))
        wT.append((
            np.ascontiguousarray(Wq[gc].T).astype(f16),
            np.ascontiguousarray(Wk[gc].T).astype(f16),
            np.ascontiguousarray(Wv[gc].T).astype(f16),
            woT2,
        ))
    ident = np.eye(128, dtype=f16)
    in_maps = []
    for c in range(NCORES):
        b, g = c // 4, c % 4
        qT, kT, vT = xT[b]
        wq, wk, wv, wo2 = wT[g]
        in_maps.append({
            "xqT": qT, "xkT": kT, "xvT": vT,
            "wqT": wq, "wkT": wk, "wvT": wv, "woT2": wo2,
            "ident": ident,
        })
    return in_maps


def kernel(query, key, value, Wq, Wk, Wv, Wo):
    query = np.asarray(query, dtype=np.float32)
    key = np.asarray(key, dtype=np.float32)
    value = np.asarray(value, dtype=np.float32)
    Wq = np.asarray(Wq, dtype=np.float32)
    Wk = np.asarray(Wk, dtype=np.float32)
    Wv = np.asarray(Wv, dtype=np.float32)
    Wo = np.asarray(Wo, dtype=np.float32)

    nc = _get_nc()
    in_maps = _shard_inputs(query, key, value, Wq, Wk, Wv, Wo)
    res = run_bass_kernel_spmd(nc, in_maps, core_ids=list(range(NCORES)))

    out = np.zeros((B, S, E), dtype=np.float32)
    for c in range(NCORES):
        out[c // 4] += res.results[c]["out"].astype(np.float32)
    return out


# revision 72
# speedup vs baseline: 1.0143x; 1.0017x over previous
"""Trainium2 Bass kernel for nn_MultiHeadAttention (B=2, S=2048, E=1024, H=16).

Sharding: 8 NeuronCores = data-parallel over the 2 batches x tensor-parallel
over the 16 heads in 4 groups of 4 heads (Wq/Wk/Wv split column-wise, Wo
row-wise).  Each core computes a full-[S, E] partial of its batch's output;
the host sums the 4 head-group partials per batch.

Per-core device algorithm (S.T orientation feeds a flipped P@V):
  Q.T/K.T[n, s] = (wT chunk).T @ xT chunk       e/m-outer projections
  S.T_h[k, q]   = (K_h.T chunk).T @ Q_h.T       row-packed head pairs (d=64)
  P.T           = exp(S.T / 8)                  fp16, one ACT op per (k, pair)
  O[q, d|sum]   = (P.T chunk).T @ [V_h | 1]     FLIPPED: P.T [128k,128q] is the
                                                stationary, [V|ones] [128k,65]
                                                moving -> 65-col outputs, with
                                                the softmax denominator landing
                                                in column 64 (per-partition!)
  O_n           = O * recip(col 64)             one DVE tensor_scalar per tile
  O.T           = PE-transpose(O_n)             53ns/tile, restores [d, q]
  out[m, :]     = sum_pair (oT2 chunk).T @ woT2 contract-128 output projection

The flip + contract-128 out-proj cut PE matmul time ~27%; exp on the
Activation engine (~133us) becomes the critical resource, so the instruction
stream is interleaved to keep it saturated: each (qc, pair) "window" weaves
the NEXT window's scores+exp with THIS window's PV, plus V/Q-projection,
out-projection, and transpose filler work sized to the Act-engine pace.

dtypes: HBM traffic fp16 (in+out); scores fp32r; P/V/O/Wo fp16; accum fp32.
"""

import numpy as np
from contextlib import ExitStack

import ml_dtypes

import concourse.bass as bass
import concourse.mybir as mybir
import concourse.tile as tile
from concourse.tile import ScopedClock
from concourse.bass_utils import run_bass_kernel_spmd

# ---------------------------------------------------------------------------
# Workarounds for the walrus build on this stack, which rejects more than ONE
# semaphore wait per instruction ("Too many sync wait commands").
# ---------------------------------------------------------------------------
_orig_commit_instruction = tile.TileContext._commit_instruction


def _commit_instruction(self, inst, lazy_reg_writes=True):
    si = getattr(inst, "sync_info", None)
    if si is not None and si.on_wait and len(si.on_wait) > 1:
        waits = list(si.on_wait)
        for w in waits[:-1]:
            nop = mybir.InstNoOp(
                name=self.nc.get_next_instruction_name(),
                ins=[], outs=[], engine=inst.engine,
            )
            nop.bass_nofuse = True
            nop.sync_info = mybir.SyncInfo(on_wait=[w], on_update=[])
            _orig_commit_instruction(self, nop, lazy_reg_writes=False)
        inst.sync_info = mybir.SyncInfo(
            on_wait=[waits[-1]], on_update=list(si.on_update or [])
        )
    return _orig_commit_instruction(self, inst, lazy_reg_writes)


def _drain_and_barrier(self, tick_clock, wait_clock):
    nc = self.nc
    drain_inst = nc.sync.drain()
    wait_clock.add_sem_waits(
        drain_inst.ins, ScopedClock({None: tick_clock.global_clock})
    )
    si = drain_inst.ins.sync_info
    waits = list(si.on_wait) if si and si.on_wait else []
    if len(waits) > 1:
        drain_inst.ins.sync_info = mybir.SyncInfo(
            on_wait=waits[:1], on_update=list(si.on_update or [])
        )
        for w in waits[1:]:
            extra = nc.sync.drain()
            esi = extra.ins.sync_info
            extra.ins.sync_info = mybir.SyncInfo(
                on_wait=[w],
                on_update=list(esi.on_update or []) if esi else [],
            )
    nc.all_engine_barrier()
    assert self.sems is not None
    popped = nc._tile_sem_poison_stack.pop()
    assert popped is self._sem_poison
    nc.clear_and_free_semaphores(list(self.sems.allocated().values()))
    nc.all_engine_barrier()


def _apply_tilefix():
    tile.TileContext._commit_instruction = _commit_instruction
    tile.TileContext._drain_and_barrier = _drain_and_barrier


_apply_tilefix()

# ---------------------------------------------------------------------------
# Problem constants (hardcoded)
# ---------------------------------------------------------------------------
B, S, E, H = 2, 2048, 1024, 16
HC, D = 4, 64              # heads per core, head dim
NCORES = 8
NE = E // 128              # 8  e-chunks (projection contraction)
NK = S // 128              # 16 k-chunks
NQC = S // 512             # 4  512-wide q windows
NM = S // 128              # 16 output row chunks

F32 = mybir.dt.float32
F32R = mybir.dt.float32r
FP16 = mybir.dt.float16
EXP = mybir.ActivationFunctionType.Exp
MULT = mybir.AluOpType.mult


def build(ptbufs=20, xdt=FP16, qkdt=mybir.dt.float32r):
    nc = bass.Bass()
    xqT = nc.dram_tensor("xqT", [E, S], xdt, kind="ExternalInput")
    xkT = nc.dram_tensor("xkT", [E, S], xdt, kind="ExternalInput")
    xvT = nc.dram_tensor("xvT", [E, S], xdt, kind="ExternalInput")
    wqT = nc.dram_tensor("wqT", [E, 256], xdt, kind="ExternalInput")
    wkT = nc.dram_tensor("wkT", [E, 256], xdt, kind="ExternalInput")
    wvT = nc.dram_tensor("wvT", [E, 256], xdt, kind="ExternalInput")
    woT2 = nc.dram_tensor("woT2", [128, 2 * E], FP16, kind="ExternalInput")
    ident = nc.dram_tensor("ident", [128, 128], FP16, kind="ExternalInput")
    out = nc.dram_tensor("out", [S, E], FP16, kind="ExternalOutput")

    with tile.TileContext(nc) as tc, ExitStack() as ctx:
        consts = ctx.enter_context(tc.tile_pool(name="consts", bufs=1))
        wpool = ctx.enter_context(tc.tile_pool(name="w", bufs=1))
        actpool = ctx.enter_context(tc.tile_pool(name="acts", bufs=1))
        xkvpool = ctx.enter_context(tc.tile_pool(name="xkv", bufs=10))
        xqpool = ctx.enter_context(tc.tile_pool(name="xq", bufs=4))

        # preload the exp table before the hot loop
        dummy = consts.tile([1, 8], F32)
        nc.vector.memset(dummy[:], 0.0)
        nc.scalar.activation(dummy[:], dummy[:], EXP)

        wq_sb = wpool.tile([128, NE, 256], xdt)
        wk_sb = wpool.tile([128, NE, 256], xdt)
        wv_sb = wpool.tile([128, NE, 256], xdt)
        wo2_sb = wpool.tile([128, 2, E], FP16)
        id_sb = wpool.tile([128, 128], FP16)

        # per-pair tensors: tile-granular dependency tracking means a read
        # waits on every writer emitted so far, so pair0's scores must not
        # share a tile with pair1's (later-copied) projections
        qT_sb = [actpool.tile([128, S], qkdt, name=f"qT{p}") for p in range(2)]
        kT_sb = [actpool.tile([128, S], qkdt, name=f"kT{p}") for p in range(2)]
        v_sb = actpool.tile([128, NK, HC, 65], FP16)   # [s%128, k, h, V_h|one]
        oT2_sb = actpool.tile([128, 2, S], FP16)       # [(2 heads x d), pair, q]

        # ---- DMA program: ordered by first-use ----
        nc.sync.dma_start(wk_sb[:], wkT.rearrange("(ec p) n -> p ec n", p=128))
        nc.sync.dma_start(wq_sb[:], wqT.rearrange("(ec p) n -> p ec n", p=128))
        xk = []
        for e in range(NE):
            t = xkvpool.tile([128, S], xdt, tag="xc", name=f"xk{e}")
            nc.sync.dma_start(t[:], xkT[e * 128:(e + 1) * 128, :])
            xk.append(t)
        # xq arrives in per-qc column blocks (ONE descriptor each: HWDGE
        # descriptor-gen at ~625ns/DMA would otherwise pace the 364ns slices)
        xqv = xqT.rearrange("(ec p) s -> p ec s", p=128)
        xq = [None] * NQC
        for qc in [0]:
            t = xqpool.tile([128, NE, 512], xdt, tag="xq", name=f"xq{qc}")
            nc.sync.dma_start(t[:], xqv[:, :, qc * 512:(qc + 1) * 512])
            xq[qc] = t
        xv = []
        for e in range(NE):
            t = xkvpool.tile([128, S], xdt, tag="xc", name=f"xv{e}")
            nc.sync.dma_start(t[:], xvT[e * 128:(e + 1) * 128, :])
            xv.append(t)
        # wv rides behind xv (first use ~32us, xv gates the V projections)
        nc.sync.dma_start(wv_sb[:], wvT.rearrange("(ec p) n -> p ec n", p=128))
        for qc in range(1, NQC):
            if qc == 2:
                # wo2/ident ride between xq1 and xq2 (first use ~55us); they
                # must not delay xq1, which gates the qc1 Q-projection
                nc.sync.dma_start(
                    wo2_sb[:], woT2.rearrange("p (two e) -> p two e", two=2))
                nc.sync.dma_start(id_sb[:], ident[0:128, :])
            t = xqpool.tile([128, NE, 512], xdt, tag="xq", name=f"xq{qc}")
            nc.sync.dma_start(t[:], xqv[:, :, qc * 512:(qc + 1) * 512])
            xq[qc] = t

        # ones column for the fused rowsum (PV moving operand col 64)
        nc.vector.memset(v_sb[:, :, :, 64:65], 1.0)

        # ---- steady-state pools (psS first: it coexists with prefix psA) ----
        psS = ctx.enter_context(tc.tile_pool(name="psS", bufs=2, space="PSUM"))
        ppool = ctx.enter_context(tc.tile_pool(name="pT", bufs=ptbufs))
        npool = ctx.enter_context(tc.tile_pool(name="nT", bufs=6))
        rpool = ctx.enter_context(tc.tile_pool(name="rt", bufs=4))
        stpool = ctx.enter_context(tc.tile_pool(name="stage", bufs=4))
        scpool = ctx.enter_context(tc.tile_pool(name="oscr", bufs=2))

        def emit_scores_exp(qc, pair, k):
            qs = slice(qc * 512, (qc + 1) * 512)
            ks = slice(k * 128, (k + 1) * 128)
            ps_s = psS.tile([128, 1024], F32, tag="ss", name=f"ss{qc}_{pair}_{k}")
            nc.tensor.matmul(ps_s[:, 0:512],
                             kT_sb[pair][0:64, ks], qT_sb[pair][0:64, qs],
                             start=True, stop=True)
            nc.tensor.matmul(ps_s[:, 512:1024],
                             kT_sb[pair][64:128, ks], qT_sb[pair][64:128, qs],
                             start=True, stop=True)
            pT = ppool.tile([128, 1024], FP16, tag="pt", name=f"pt{qc}_{pair}_{k}")
            nc.scalar.activation(pT[:], ps_s[:], EXP, scale=0.125)
            return pT

        def emit_pv(pair, k, pT, ovv, qts=(0, 1, 2, 3)):
            # ovv: ps_o viewed as [128, qt, h2, 128]; O in cols 0:64, sum col
            # 64. qt-major so qt 0-1 fill PSUM bank0 and qt 2-3 bank1 (the
            # last window is drained per qt-half). PSUM zero regions are 2KB:
            # start=True pending-zeroes the WHOLE bank, so only the first
            # region per bank may start the group; the other regions' k==0
            # writes land on pending-zero bytes and write-through (hardware
            # lazy-zero semantics).
            for qt in qts:
                for h2 in range(2):
                    h = pair * 2 + h2
                    nc.tensor.matmul(
                        ovv[:, qt, h2, 0:65],
                        pT[:, h2 * 512 + qt * 128:h2 * 512 + (qt + 1) * 128],
                        v_sb[:, k, h, 0:65],
                        start=(k == 0 and h2 == 0 and qt in (0, 2)),
                        stop=(k == NK - 1),
                        skip_group_check=True,
                    )

        def emit_vproj(k):
            t = psM.tile([128, 512], F32, tag="mi", name=f"vp{k}")
            for e in range(NE):
                nc.tensor.matmul(
                    t[:, 0:256],
                    xv[e][:, k * 128:(k + 1) * 128],
                    wv_sb[:, e, :],
                    start=(e == 0), stop=(e == NE - 1),
                )
            nc.vector.tensor_copy(
                v_sb[:, k, :, 0:64],
                t[:, 0:256].rearrange("p (h c) -> p h c", h=HC))

        def emit_qproj(qc, nch):
            t = psM.tile([128, 512], F32, tag="mi", name=f"qp{qc}_{nch}")
            for e in range(NE):
                nc.tensor.matmul(
                    t[:],
                    wq_sb[:, e, nch * 128:(nch + 1) * 128],
                    xq[qc][:, e, :],
                    start=(e == 0), stop=(e == NE - 1),
                )
            nc.vector.tensor_copy(qT_sb[nch][:, qc * 512:(qc + 1) * 512], t[:])

        def emit_norm(qc, pair, ovv, qts=(0, 1, 2, 3), split=False):
            # Mid-stream: evacuate ps_o to SBUF in ONE copy so the next
            # window's PV (which reuses the psOV ring slot) waits ~1.2us
            # instead of ~3us of piecemeal norm reads.
            if not split:
                osc = scpool.tile([128, 1024], F32, tag="os",
                                  name=f"os{qc}_{pair}")
                nc.vector.tensor_copy(osc[:], ovv.rearrange(
                    "p q h c -> p (q h c)"))
                ovv = osc.rearrange("p (q h c) -> p q h c", q=4, h=2)
            # reciprocal of the fused rowsums (col 64 of each 128-col region)
            n = len(qts)
            rt = rpool.tile([128, 2 * n], F32, tag="rt",
                            name=f"rt{qc}_{pair}_{qts[0]}")
            nc.vector.reciprocal(
                rt[:].rearrange("p (q h) -> p q h", q=n),
                ovv[:, qts[0]:qts[0] + n, :, 64])
            nTs = {}
            for i, qt in enumerate(qts):
                nT = npool.tile([128, 128], FP16, tag="nt",
                                name=f"nt{qc}_{pair}_{qt}")
                for h2 in range(2):
                    if split and qt >= 2:
                        # tail only: whole tiles on the idle Act engine
                        # (mixed engines per tile serialize on WAW sems)
                        nc.scalar.mul(nT[:, h2 * 64:(h2 + 1) * 64],
                                      ovv[:, qt, h2, 0:64],
                                      rt[:, 2 * i + h2:2 * i + h2 + 1])
                    else:
                        nc.vector.tensor_scalar(
                            nT[:, h2 * 64:(h2 + 1) * 64],
                            ovv[:, qt, h2, 0:64],
                            rt[:, 2 * i + h2:2 * i + h2 + 1],
                            None, MULT)
                nTs[qt] = nT
            return nTs

        def emit_transpose(qc, pair, qt, nT, pool=None):
            # pool=psS in the tail: scores are done, its slots are idle, and
            # keeping transposes out of the psM ring stops them serializing
            # behind out-proj tiles awaiting their stage copies.
            pool = pool or psM
            tag = {id(psS): "ss", id(psOV): "ov"}.get(id(pool), "mi")
            psT = pool.tile([128, 128], FP16, tag=tag, name=f"tp{qc}_{pair}_{qt}")
            nc.tensor.transpose(psT[:], nT[:], id_sb[:])
            nc.vector.tensor_copy(
                oT2_sb[:, pair, qc * 512 + qt * 128:qc * 512 + (qt + 1) * 128],
                psT[:])

        def emit_outproj(m, split=False, pool=None):
            pool = pool or psM
            tag = {id(psS): "ss", id(psOV): "ov"}.get(id(pool), "mi")
            stage = stpool.tile([128, E], FP16, tag="st", name=f"st{m}")
            for j in range(2):
                t = pool.tile([128, 512], F32, tag=tag, name=f"op{m}_{j}")
                for pair in range(2):
                    nc.tensor.matmul(
                        t[:],
                        oT2_sb[:, pair, m * 128:(m + 1) * 128],
                        wo2_sb[:, pair, j * 512:(j + 1) * 512],
                        start=(pair == 0), stop=(pair == 1),
                    )
                if split and j == 1:
                    nc.scalar.copy(stage[:, 512:1024], t[:])
                else:
                    nc.vector.tensor_copy(stage[:, j * 512:(j + 1) * 512], t[:])
            if split:
                nc.sync.dma_start(out[m * 128:(m + 1) * 128, :], stage[:])
            else:
                nc.gpsimd.dma_start(out[m * 128:(m + 1) * 128, :], stage[:])

        # ---- prefix: K/Q projections, nch-split so pair0 lands fast ----
        # psA (4 banks) coexists with psS (4 banks); pair0's K tiles project
        # and copy out first, Q-qc0 goes through psS slots, and the first
        # scores fire while pair1's K projection finishes in the background.
        pts = {}
        cur = []
        vdone = 0
        with tc.tile_pool(name="psA", bufs=4, space="PSUM") as psA:
            kt0 = [psA.tile([128, 512], F32, tag="mm", name=f"pk0_{m}")
                   for m in range(4)]
            kt1 = [psA.tile([128, 512], F32, tag="mm", name=f"pk1_{m}")
                   for m in range(4)]
            for e in range(NE):
                for m in range(4):
                    nc.tensor.matmul(
                        kt0[m][:], wk_sb[:, e, 0:128],
                        xk[e][:, m * 512:(m + 1) * 512],
                        start=(e == 0), stop=(e == NE - 1))
            for m in range(4):
                nc.vector.tensor_copy(
                    kT_sb[0][:, m * 512:(m + 1) * 512], kt0[m][:])
            # Q-qc0 pair0 chases the xq0 DMAs immediately after K-pair0 (the
            # PE stream is in-order, so nothing may sit between them); all of
            # pair1's projection work weaves into the first scores' shadows.
            tq0 = psS.tile([128, 512], F32, tag="ss", name="pq0_0")
            # tq1 must NOT take a psS ring slot: its reader (the qT1 copy)
            # lands after the early scores, which would deadlock their
            # tile allocations. The psA ring's kt0[0] slot frees early.
            tq1 = psA.tile([128, 512], F32, tag="mm", name="pq0_1")
            for e in range(NE):
                nc.tensor.matmul(
                    tq0[:], wq_sb[:, e, 0:128], xq[0][:, e, :],
                    start=(e == 0), stop=(e == NE - 1))
            nc.vector.tensor_copy(qT_sb[0][:, 0:512], tq0[:])
            for k in range(6):
                cur.append(emit_scores_exp(0, 0, k))
                if k < 4:     # K-pair1, two e-chunks per score shadow
                    for e in (2 * k, 2 * k + 1):
                        for m in range(4):
                            nc.tensor.matmul(
                                kt1[m][:], wk_sb[:, e, 128:256],
                                xk[e][:, m * 512:(m + 1) * 512],
                                start=(e == 0), stop=(e == NE - 1))
                elif k < 6:   # then Q-pair1, four e-chunks per shadow
                    for e in range(4 * (k - 4), 4 * (k - 3)):
                        nc.tensor.matmul(
                            tq1[:], wq_sb[:, e, 128:256], xq[0][:, e, :],
                            start=(e == 0), stop=(e == NE - 1))
            for m in range(4):
                nc.vector.tensor_copy(
                    kT_sb[1][:, m * 512:(m + 1) * 512], kt1[m][:])
            for k in range(6, 8):
                cur.append(emit_scores_exp(0, 0, k))
                if k == 6:
                    nc.vector.tensor_copy(qT_sb[1][:, 0:512], tq1[:])
        psOV = ctx.enter_context(tc.tile_pool(name="psOV", bufs=1, space="PSUM"))
        psM = ctx.enter_context(tc.tile_pool(name="psM", bufs=2, space="PSUM"))

        for k in range(8, NK):
            cur.append(emit_scores_exp(0, 0, k))
            if k >= 9 and vdone < 5:   # xv DMAs have landed by these shadows
                emit_vproj(vdone)
                vdone += 1
            elif k == 14:
                emit_qproj(1, 0)
            elif k == 15:
                emit_qproj(1, 1)
        pts[(0, 0)] = cur

        seq = [(qc, pair) for qc in range(NQC) for pair in range(2)]
        trans_pending = None   # (qc, pair, nTs) awaiting transpose weave
        for i, (qc, pair) in enumerate(seq):
            nxt = seq[i + 1] if i + 1 < len(seq) else None
            ps_o = psOV.tile([128, 1024], F32, tag="ov", name=f"ov{qc}_{pair}")
            ovv = ps_o.rearrange("p (q h c) -> p q h c", q=4, h=2)
            cur_pts = pts.pop((qc, pair))
            nxt_pts = [] if nxt else None
            op_ms = list(range((qc - 1) * 4, qc * 4)) if (pair == 0 and qc >= 1) else []
            last_qts = (0, 1, 2, 3)
            for k in range(NK):
                if nxt:
                    nxt_pts.append(emit_scores_exp(nxt[0], nxt[1], k))
                if trans_pending and k in (2, 4, 6, 8):
                    tqc, tpair, tnTs = trans_pending
                    emit_transpose(tqc, tpair, k // 2 - 1, tnTs[k // 2 - 1])
                    if k == 8:
                        trans_pending = None
                if vdone < NK:  # remaining V-proj tiles, just-in-time
                    emit_vproj(vdone)
                    vdone += 1
                emit_pv(pair, k, cur_pts[k], ovv, qts=last_qts)
                if op_ms and k % 4 == 2:
                    emit_outproj(op_ms[k // 4])
                if pair == 0 and 1 <= qc < NQC - 1 and k in (8, 12):
                    # Q projection for the next qc, before its scores appear
                    emit_qproj(qc + 1, (k - 8) // 4)
            if nxt:
                pts[nxt] = nxt_pts
                nTs = emit_norm(qc, pair, ovv)
                trans_pending = (qc, pair, nTs)
            else:
                # tail drain: norm (split across DVE + the idle Act engine),
                # then per-q-tile transpose + out-proj chains distributed
                # over all three PSUM rings so they pipeline
                mb = (NQC - 1) * 4
                # evacuate ps_o in two parallel halves (DVE + Act), then each
                # engine normalizes only the half it wrote (no cross-tile WAW)
                oscA = scpool.tile([128, 512], F32, tag="os", name="oscA")
                nc.vector.tensor_copy(oscA[:], ps_o[:, 0:512])
                oscB = scpool.tile([128, 512], F32, tag="os", name="oscB")
                nc.scalar.copy(oscB[:], ps_o[:, 512:1024])
                rtA = rpool.tile([128, 4], F32, tag="rt", name="rtA")
                rtB = rpool.tile([128, 4], F32, tag="rt", name="rtB")
                vA = oscA.rearrange("p (q h c) -> p q h c", q=2, h=2)
                vB = oscB.rearrange("p (q h c) -> p q h c", q=2, h=2)
                nc.vector.reciprocal(
                    rtA[:].rearrange("p (q h) -> p q h", q=2), vA[:, :, :, 64])
                nc.vector.reciprocal(
                    rtB[:].rearrange("p (q h) -> p q h", q=2), vB[:, :, :, 64])
                nTs = {}
                for qt in range(4):
                    nT = npool.tile([128, 128], FP16, tag="nt", name=f"ntt{qt}")
                    v, rt_, lq = (vA, rtA, qt) if qt < 2 else (vB, rtB, qt - 2)
                    for h2 in range(2):
                        if qt < 2:
                            nc.vector.tensor_scalar(
                                nT[:, h2 * 64:(h2 + 1) * 64],
                                v[:, lq, h2, 0:64],
                                rt_[:, 2 * lq + h2:2 * lq + h2 + 1], None, MULT)
                        else:
                            nc.scalar.mul(
                                nT[:, h2 * 64:(h2 + 1) * 64],
                                v[:, lq, h2, 0:64],
                                rt_[:, 2 * lq + h2:2 * lq + h2 + 1])
                    nTs[qt] = nT
                emit_transpose(qc, pair, 0, nTs[0], pool=psS)
                emit_transpose(qc, pair, 1, nTs[1], pool=psS)
                emit_transpose(qc, pair, 2, nTs[2], pool=psOV)
                emit_transpose(qc, pair, 3, nTs[3], pool=psOV)
                emit_outproj(mb + 0, split=True, pool=psM)
                emit_outproj(mb + 1, split=True, pool=psS)
                emit_outproj(mb + 2, split=True, pool=psM)
                emit_outproj(mb + 3, split=True, pool=psS)

    return nc


_NC_CACHE = {}


def _get_nc():
    if "nc" not in _NC_CACHE:
        _NC_CACHE["nc"] = build()
    return _NC_CACHE["nc"]


def _shard_inputs(query, key, value, Wq, Wk, Wv, Wo):
    """Host-side sharding + layout prep: core c = (batch c//4, head-group c%4)."""
    f16 = np.float16
    xT = []
    for b in range(B):
        xT.append((
            np.ascontiguousarray(query[b].T).astype(f16),
            np.ascontiguousarray(key[b].T).astype(f16),
            np.ascontiguousarray(value[b].T).astype(f16),
        ))
    wT = []
    for g in range(4):
        gc = slice(g * 256, (g + 1) * 256)
        wo_g = Wo[:, gc].T.astype(f16)            # [256, E]
        woT2 = np.ascontiguousarray(
            wo_g.reshape(2, 128, E).transpose(1, 0, 2).reshape(128, 2 * E))
        wT.append((
            np.ascontiguousarray(Wq[gc].T).astype(f16),
            np.ascontiguousarray(Wk[gc].T).astype(f16),
            np.ascontiguousarray(Wv[gc].T).astype(f16),
            woT2,
        ))
    ident = np.eye(128, dtype=f16)
    in_maps = []
    for c in range(NCORES):
        b, g = c // 4, c % 4
        qT, kT, vT = xT[b]
        wq, wk, wv, wo2 = wT[g]
        in_maps.append({
            "xqT": qT, "xkT": kT, "xvT": vT,
            "wqT": wq, "wkT": wk, "wvT": wv, "woT2": wo2,
            "ident": ident,
        })
    return in_maps


def kernel(query, key, value, Wq, Wk, Wv, Wo):
    query = np.asarray(query, dtype=np.float32)
    key = np.asarray(key, dtype=np.float32)
    value = np.asarray(value, dtype=np.float32)
    Wq = np.asarray(Wq, dtype=np.float32)
    Wk = np.asarray(Wk, dtype=np.float32)
    Wv = np.asarray(Wv, dtype=np.float32)
    Wo = np.asarray(Wo, dtype=np.float32)

    nc = _get_nc()
    in_maps = _shard_inputs(query, key, value, Wq, Wk, Wv, Wo)
    res = run_bass_kernel_spmd(nc, in_maps, core_ids=list(range(NCORES)))

    out = np.zeros((B, S, E), dtype=np.float32)
    for c in range(NCORES):
        out[c // 4] += res.results[c]["out"].astype(np.float32)
    return out
